# revision 27
# baseline (speedup 1.0000x reference)
"""Trainium2 Bass kernel for nn_Att_PD_layer1 (ragged dual-FCNet attention logits
+ ragged pad + masked softmax), data-parallel over 8 NeuronCores.

Contract: kernel(**inputs) takes the FULL unsharded inputs and returns the FULL
[B, 4, maxlen, K] output. Sharding: 2 whole questions per core (balanced
pairing by token*valid-box rows). Only (token, valid-box) rows go through the
GEMMs.

v2: layer-1 v-GEMMs run as fp8e4 DoubleRow matmuls (2 k-tiles of 128 per
instruction; measured ~1.7x bf16 throughput on HW). Weights are
host-quantized at x64 scale; the 1/64 rescale folds into the layer-2
h-weights (leaky-relu is positively homogeneous) and into the sigmoid
drain's scale. All PSUM drains run on ACT using one activation table
(Sigmoid / Prelu / Identity share a set; Pool cannot read PSUM on HW).
v arrives host-pre-transposed fp8 in chunk-major layout (no DMA
transposes); q arrives host-pre-transposed bf16. The ragged scatter
transposes the logit row via tiny PE matmuls instead of a DRAM round trip.

Accuracy: fp8 on layer 1 only costs ~1.5e-2 rel err (gate 2e-2). Measured
dead ends: fp8 on layer 2 or on the final hg.w_lin dot blows the budget
(2.2-3.5e-2) even with split-weight compensation, because DoubleRow is only
~1.7x (not 4x) on real HW, split costs as much as it saves.
"""
import sys
import os

sys.path.insert(0, "/opt/trn_rl_repo")
# this axon env has no NTFF profiling hook; a stray BASS_TRACE=1 would crash
os.environ["BASS_NEVER_TRACE"] = "1"

import numpy as np
import ml_dtypes
from contextlib import ExitStack

import concourse.bass as bass
import concourse.tile as tile
from concourse import bacc, mybir
from concourse.bass_interp import get_hw_module
from concourse import bass_utils

F32 = mybir.dt.float32
BF16 = mybir.dt.bfloat16
FP8 = mybir.dt.float8e4
AF = mybir.ActivationFunctionType
ALU = mybir.AluOpType
DR = mybir.MatmulPerfMode.DoubleRow
BF = ml_dtypes.bfloat16
E4 = ml_dtypes.float8_e4m3fn

B, G, ML, K = 16, 4, 16, 36
VD, QD, NH = 1024, 1024, 1024
NEG_SLOPE = 0.01
SW = 64.0            # fp8 weight scale

TPC = 112                 # max tokens per core
ROWS = 1792               # max packed (token, valid-box) rows per core
RCNS = (512, 512, 512, 256)   # rows per chunk (128-multiples)
NCHK = ROWS // 128        # 14 scatter column-chunks
NCORES = 8

LAST_RESULT = None
_CACHE = {}
_TIMING_REPS = None       # when set, wraps the main body in a For_i (timing only)
_TIMING_NO_DRAIN = False  # timing probe: emit matmul stream only (garbage output)
_TIMING_UNROLL = 1        # bodies per For_i iteration (timing only)


def _build_program():
    nc = bacc.Bacc("TRN2", target_bir_lowering=False, debug=False,
                   num_devices=NCORES)

    # ---- DRAM I/O (per-core shapes; same program on all 8 cores) ----
    # v pre-transposed+quantized on host: chunk-major [128, sum(8*rcn)]
    vbT = nc.dram_tensor("vbT", [128, 8 * ROWS], FP8, kind="ExternalInput")
    qTd = nc.dram_tensor("qTd", [128, 8, 128], BF16, kind="ExternalInput")
    w1q = nc.dram_tensor("w1q", [128, 8, NH], BF16, kind="ExternalInput")
    wg1q = nc.dram_tensor("wg1q", [128, 8, NH], BF16, kind="ExternalInput")
    w1v8 = nc.dram_tensor("w1v8", [128, 8, NH], FP8, kind="ExternalInput")
    wg1v8 = nc.dram_tensor("wg1v8", [128, 8, NH], FP8, kind="ExternalInput")
    # layer-2 weights in mc-major layout [p, mc, kc, 128] so each mc-slice is
    # a contiguous early DMA
    w2s = nc.dram_tensor("w2s", [128, 8, 8, 128], BF16, kind="ExternalInput")
    wg2 = nc.dram_tensor("wg2", [128, 8, 8, 128], BF16, kind="ExternalInput")
    wlin = nc.dram_tensor("wlin", [128, 8], BF16, kind="ExternalInput")
    b1x64 = nc.dram_tensor("b1x64", [128, 8], F32, kind="ExternalInput")
    bg1d = nc.dram_tensor("bg1d", [128, 8], F32, kind="ExternalInput")
    b2d = nc.dram_tensor("b2d", [128, 8], F32, kind="ExternalInput")
    bg2d = nc.dram_tensor("bg2d", [128, 8], F32, kind="ExternalInput")
    blind = nc.dram_tensor("blind", [1, 1], F32, kind="ExternalInput")
    seld = nc.dram_tensor("seld", [128, ROWS], BF16, kind="ExternalInput")
    maskd = nc.dram_tensor("maskd", [128, K], F32, kind="ExternalInput")
    escatd = nc.dram_tensor("escatd", [128, NCHK, 128], F32, kind="ExternalInput")
    mscatd = nc.dram_tensor("mscatd", [128, NCHK, K], F32, kind="ExternalInput")
    ones11d = nc.dram_tensor("ones11d", [1, 1], F32, kind="ExternalInput")

    outd = nc.dram_tensor("outd", [128, K], F32, kind="ExternalOutput")

    with tile.TileContext(nc) as tc:
        with ExitStack() as ctx:
            wpool = ctx.enter_context(tc.tile_pool(name="weights", bufs=1))
            const = ctx.enter_context(tc.tile_pool(name="const", bufs=1))
            vload = ctx.enter_context(tc.tile_pool(name="vload", bufs=2))
            acts = ctx.enter_context(tc.tile_pool(name="acts", bufs=2))
            small = ctx.enter_context(tc.tile_pool(name="small", bufs=3))
            psum = ctx.enter_context(tc.tile_pool(name="psum", bufs=4, space="PSUM"))
            psq = ctx.enter_context(tc.tile_pool(name="psq", bufs=2, space="PSUM"))
            pspad = ctx.enter_context(tc.tile_pool(name="pspad", bufs=1, space="PSUM"))

            def load_w(dt_, tag, dtype, halves=False):
                t = wpool.tile([128, 8, NH], dtype, tag=tag)
                if halves:
                    # two DMAs so the first U fout-half can start sooner
                    nc.gpsimd.dma_start(t[:, :, :512], dt_.ap()[:, :, :512])
                    nc.gpsimd.dma_start(t[:, :, 512:], dt_.ap()[:, :, 512:])
                else:
                    nc.gpsimd.dma_start(t[:], dt_.ap())
                return t

            # startup-critical loads first (DMA priority follows emission
            # order): U-path weights, then the fp8 layer-1 weights.
            qT = const.tile([128, 8, 128], BF16)
            nc.gpsimd.dma_start(qT[:], qTd.ap())
            w1q_t = load_w(w1q, "w1q", BF16, halves=True)
            w1v_t = load_w(w1v8, "w1v8", FP8)
            wg1q_t = load_w(wg1q, "wg1q", BF16, halves=True)
            wg1v_t = load_w(wg1v8, "wg1v8", FP8)

            wlin_t = const.tile([128, 8], BF16)
            nc.gpsimd.dma_start(wlin_t[:], wlin.ap())
            b1x64_t = const.tile([128, 8], F32)
            nc.gpsimd.dma_start(b1x64_t[:], b1x64.ap())
            bg1_t = const.tile([128, 8], F32)
            nc.gpsimd.dma_start(bg1_t[:], bg1d.ap())
            b2_t = const.tile([128, 8], F32)
            nc.gpsimd.dma_start(b2_t[:], b2d.ap())
            bg2_t = const.tile([128, 8], F32)
            nc.gpsimd.dma_start(bg2_t[:], bg2d.ap())
            blin_t = const.tile([1, 1], F32)
            nc.gpsimd.dma_start(blin_t[:], blind.ap())
            ones11_t = const.tile([1, 1], F32)
            nc.gpsimd.dma_start(ones11_t[:], ones11d.ap())
            sel_t = const.tile([128, ROWS], BF16)
            nc.gpsimd.dma_start(sel_t[:], seld.ap())

            # ---- U = (q @ Wq + b) * 64  [t(128 part), fout(1024)] bf16
            def compute_u(name, wq, bias_t):
                ut = const.tile([128, NH], BF16, tag=f"U{name}")
                for nchunk in range(2):
                    ps = psq.tile([128, 512], F32, tag="aux")
                    for kc in range(8):
                        nc.tensor.matmul(
                            ps[:], qT[:, kc, :],
                            wq[:, kc, nchunk * 512:(nchunk + 1) * 512],
                            start=(kc == 0), stop=(kc == 7))
                    if bias_t is None:
                        nc.scalar.activation(
                            ut[:, nchunk * 512:(nchunk + 1) * 512], ps[:],
                            AF.Identity, scale=SW)
                    else:
                        for j in range(4):
                            c = nchunk * 4 + j
                            nc.scalar.activation(
                                ut[:, c * 128:(c + 1) * 128],
                                ps[:, j * 128:(j + 1) * 128],
                                AF.Identity, scale=SW, bias=bias_t[:, c:c + 1])
                return ut

            U = {"h": compute_u("h", w1q_t, b1x64_t),
                 "g": compute_u("g", wg1q_t, None)}

            # logits row accumulator [1, ROWS] f32 (partition 0)
            lrow = const.tile([1, ROWS], F32)
            padded_ps = pspad.tile([128, K], F32, tag="padded")

            late = {}

            def late_loads():
                # mc-major: one DMA per mc-slice so l2-g mc0 can start early
                for nm, dt_ in (("wg2", wg2), ("w2", w2s)):
                    t = wpool.tile([128, 8, 8, 128], BF16, tag=nm)
                    for mc in range(8):
                        nc.gpsimd.dma_start(t[:, mc, :, :], dt_.ap()[:, mc, :, :])
                    late[nm] = t

            def tail_loads():
                late["escat"] = const.tile([128, NCHK, 128], F32, tag="escat_t",
                                           name="escat_t")
                nc.gpsimd.dma_start(late["escat"][:], escatd.ap())
                late["mscat"] = const.tile([128, NCHK, K], F32, tag="mscat_t",
                                           name="mscat_t")
                nc.gpsimd.dma_start(late["mscat"][:], mscatd.ap())
                late["mask"] = const.tile([128, K], F32, tag="mask_t",
                                          name="mask_t")
                nc.gpsimd.dma_start(late["mask"][:], maskd.ap())

            def main_body():
                r0 = 0
                off = 0          # element offset into vbT (chunk-major)
                for rc, rcn in enumerate(RCNS):
                    vT = vload.tile([128, 8, 512], FP8)
                    nc.sync.dma_start(
                        vT[:, :, :rcn],
                        vbT.ap()[:, off:off + 8 * rcn].rearrange(
                            "p (kc r) -> p kc r", kc=8))

                    h1T = acts.tile([128, 8, 512], BF16, tag="h1T")
                    g1T = acts.tile([128, 8, 512], BF16, tag="g1T")
                    hgT = acts.tile([128, 8, 512], BF16, tag="hgT")

                    # layer 1: 4 fp8 DoubleRow matmuls + bf16 sel/U matmul
                    def l1_mm(wv, uname, mc, ps):
                        for kp in range(4):
                            nc.tensor.matmul(
                                ps[:, :rcn],
                                wv[:, 2 * kp:2 * kp + 2, mc * 128:(mc + 1) * 128],
                                vT[:, 2 * kp:2 * kp + 2, :rcn],
                                start=(kp == 0), stop=False, perf_mode=DR)
                        nc.tensor.matmul(
                            ps[:, :rcn], U[uname][:, mc * 128:(mc + 1) * 128],
                            sel_t[:, r0:r0 + rcn], start=False, stop=True)

                    # h branch: drain = parametric-relu on ACT (64x scale
                    # stays -- prelu is positively homogeneous; b1 already
                    # folded into U-h). Prelu shares the sigmoid act table.
                    for mc in range(8):
                        ps = psum.tile([128, 512], F32, tag="ps")
                        l1_mm(w1v_t, "h", mc, ps)
                        if not _TIMING_NO_DRAIN:
                            nc.scalar.activation(h1T[:, mc, :rcn], ps[:, :rcn],
                                                 AF.Prelu, alpha=NEG_SLOPE)
                    # g branch: drain = sigmoid on ACT (scale 1/64, bias bg1)
                    for mc in range(8):
                        ps = psum.tile([128, 512], F32, tag="ps")
                        l1_mm(wg1v_t, "g", mc, ps)
                        if not _TIMING_NO_DRAIN:
                            nc.scalar.activation(g1T[:, mc, :rcn], ps[:, :rcn],
                                                 AF.Sigmoid, scale=1.0 / SW,
                                                 bias=bg1_t[:, mc:mc + 1])

                    if rc == 0 and "w2" not in late:
                        late_loads()
                    w2_t, wg2_t = late["w2"], late["wg2"]

                    # layer 2 g: bf16 matmuls, sigmoid drain on ACT
                    g2T = small.tile([128, 8, 512], BF16, tag="g2T")
                    for mc in range(8):
                        ps = psum.tile([128, 512], F32, tag="ps")
                        for kc in range(8):
                            rhs = (sel_t[:, kc * 128:kc * 128 + rcn]
                                   if _TIMING_NO_DRAIN else g1T[:, kc, :rcn])
                            nc.tensor.matmul(
                                ps[:, :rcn], wg2_t[:, mc, kc, :],
                                rhs, start=(kc == 0), stop=(kc == 7))
                        if not _TIMING_NO_DRAIN:
                            nc.scalar.activation(g2T[:, mc, :rcn], ps[:, :rcn],
                                                 AF.Sigmoid,
                                                 bias=bg2_t[:, mc:mc + 1])
                    # layer 2 h: bf16 matmuls (W2/64 vs 64-scaled h1), then
                    # parametric-relu drain on ACT (Prelu shares the sigmoid
                    # act table; bias=b2), then h2*g2 on DVE
                    for mc in range(8):
                        ps = psum.tile([128, 512], F32, tag="ps")
                        for kc in range(8):
                            rhs = (sel_t[:, kc * 128:kc * 128 + rcn]
                                   if _TIMING_NO_DRAIN else h1T[:, kc, :rcn])
                            nc.tensor.matmul(
                                ps[:, :rcn], w2_t[:, mc, kc, :],
                                rhs, start=(kc == 0), stop=(kc == 7))
                        h2t = small.tile([128, 512], BF16, tag="h2t")
                        if not _TIMING_NO_DRAIN:
                            nc.scalar.activation(h2t[:, :rcn], ps[:, :rcn],
                                                 AF.Prelu,
                                                 bias=b2_t[:, mc:mc + 1],
                                                 alpha=NEG_SLOPE)
                            nc.vector.tensor_mul(hgT[:, mc, :rcn], h2t[:, :rcn],
                                                 g2T[:, mc, :rcn])

                    # final: logits[r] = hg[:, r] . wlin + blin
                    psl = psq.tile([1, 512], F32, tag="aux")
                    for kc in range(8):
                        rhs = (sel_t[:, kc * 128:kc * 128 + rcn]
                               if _TIMING_NO_DRAIN else hgT[:, kc, :rcn])
                        nc.tensor.matmul(psl[:, :rcn], wlin_t[:, kc:kc + 1],
                                         rhs, start=(kc == 0),
                                         stop=(kc == 7))
                    if not _TIMING_NO_DRAIN:
                        nc.vector.tensor_scalar_add(lrow[:, r0:r0 + rcn],
                                                    psl[:, :rcn], blin_t[:, 0:1])

                    if rc == 0 and "escat" not in late:
                        tail_loads()
                    escat_t, mscat_t, mask_t = (late["escat"], late["mscat"],
                                                late["mask"])

                    # incremental ragged scatter of finished 128-blocks:
                    # transpose lrow pieces to partitions via 1-wide matmuls,
                    # then 0/1 matmuls accumulate into padded_ps.
                    c0, ncc = r0 // 128, rcn // 128
                    r0 += rcn
                    off += 8 * rcn
                    if _TIMING_NO_DRAIN:
                        continue
                    scps = pspad.tile([128, 4], F32, tag="scps")
                    for cc in range(ncc):
                        c = c0 + cc
                        nc.tensor.matmul(
                            scps[:, cc:cc + 1],
                            lrow[0:1, c * 128:(c + 1) * 128], ones11_t[:],
                            start=True, stop=True, skip_group_check=True)
                        rhs_c = small.tile([128, K], F32, tag="rhs_c")
                        nc.vector.tensor_scalar_mul(rhs_c[:], mscat_t[:, c, :],
                                                    scps[:, cc:cc + 1])
                        nc.tensor.matmul(padded_ps[:], escat_t[:, c, :], rhs_c[:],
                                         start=(c == 0), stop=(c == NCHK - 1),
                                         skip_group_check=True)

                if _TIMING_NO_DRAIN:
                    outt = small.tile([128, K], F32, tag="outt")
                    nc.vector.memset(outt[:], 0.0)
                    nc.sync.dma_start(outd.ap(), outt[:])
                    return
                # ---- masked softmax tail (f32, exact reference semantics)
                vecm = small.tile([128, K], F32, tag="vecm")
                nc.vector.tensor_mul(vecm[:], padded_ps[:], mask_t[:])
                negmx = small.tile([128, 1], F32, tag="negmx")
                nc.vector.reduce_max(negmx[:], vecm[:], axis=mybir.AxisListType.X,
                                     negate=True)
                e = small.tile([128, K], F32, tag="e")
                nc.scalar.activation(e[:], vecm[:], AF.Exp, bias=negmx[:])
                z = small.tile([128, 1], F32, tag="z")
                nc.vector.reduce_sum(z[:], e[:], axis=mybir.AxisListType.X)
                em = small.tile([128, K], F32, tag="em")
                nc.vector.tensor_mul(em[:], e[:], mask_t[:])
                s2 = small.tile([128, 1], F32, tag="s2")
                nc.vector.reduce_sum(s2[:], em[:], axis=mybir.AxisListType.X)
                den = small.tile([128, 1], F32, tag="den")
                nc.vector.tensor_scalar_mul(den[:], z[:], 1e-13)
                nc.vector.tensor_add(den[:], den[:], s2[:])
                rec = small.tile([128, 1], F32, tag="rec")
                nc.vector.reciprocal(rec[:], den[:])
                outt = small.tile([128, K], F32, tag="outt")
                nc.vector.tensor_scalar_mul(outt[:], em[:], rec[:])
                nc.sync.dma_start(outd.ap(), outt[:])

            if _TIMING_REPS:
                late_loads()
                tail_loads()
                with tc.For_i(0, _TIMING_REPS, 1):
                    for _ in range(_TIMING_UNROLL):
                        main_body()
            else:
                main_body()

    nc.compile()
    nc.m = get_hw_module(nc.m)
    return nc


def _pair_questions(weight):
    """Greedy balanced pairing: sort desc, pair largest with smallest."""
    order = np.argsort(-np.asarray(weight), kind="stable")
    pairs = []
    lo, hi = 0, len(order) - 1
    while lo < hi:
        pairs.append((int(order[lo]), int(order[hi])))
        lo += 1
        hi -= 1
    return pairs


def _rearrange_w(w):
    """[1024, NH] -> [128, 8, NH] with fin = kc*128 + p."""
    return np.ascontiguousarray(w.reshape(8, 128, NH).transpose(1, 0, 2))


def _rearrange_w_mc(w):
    """[1024, NH] -> [128, 8mc, 8kc, 128] with fin = kc*128+p, fout = mc*128+f."""
    return np.ascontiguousarray(
        w.reshape(8, 128, 8, 128).transpose(1, 2, 0, 3))


def kernel(v, q, box_mask, tags_attention, W1, b1, W2, b2, Wg1, bg1, Wg2, bg2,
           w_lin, b_lin):
    global LAST_RESULT
    v = np.asarray(v, dtype=np.float32)
    q = np.asarray(q, dtype=np.float32)
    box_mask = np.asarray(box_mask, dtype=np.float32)
    tags_attention = np.asarray(tags_attention)

    lengths = tags_attention.sum(-1).astype(np.int64)          # [B, G]
    qlen = lengths.sum(-1)                                     # [B]
    qstart = np.concatenate([[0], np.cumsum(qlen)[:-1]])
    valid_ks = [np.where(box_mask[b] > 0)[0] for b in range(B)]
    nval = np.array([len(vk) for vk in valid_ks])
    pairs = _pair_questions(qlen * nval)
    assert len(pairs) == NCORES
    assert max(qlen[a] + qlen[b] for a, b in pairs) <= TPC
    assert max(qlen[a] * nval[a] + qlen[b] * nval[b] for a, b in pairs) <= ROWS

    W1 = np.asarray(W1, np.float32)
    Wg1 = np.asarray(Wg1, np.float32)
    W2 = np.asarray(W2, np.float32)
    Wg2 = np.asarray(Wg2, np.float32)

    # shared (per-core identical) tensors
    wb = {
        "w1q": _rearrange_w(W1[VD:]).astype(BF),
        "wg1q": _rearrange_w(Wg1[VD:]).astype(BF),
        "w1v8": _rearrange_w(W1[:VD] * SW).astype(E4),
        "wg1v8": _rearrange_w(Wg1[:VD] * SW).astype(E4),
        "w2s": _rearrange_w_mc(W2 / SW).astype(BF),
        "wg2": _rearrange_w_mc(Wg2).astype(BF),
        "wlin": np.asarray(w_lin).reshape(8, 128).T.copy().astype(BF),
        "b1x64": (np.asarray(b1, np.float32) * SW).reshape(8, 128).T.copy(),
        "bg1d": np.asarray(bg1, np.float32).reshape(8, 128).T.copy(),
        "b2d": np.asarray(b2, np.float32).reshape(8, 128).T.copy(),
        "bg2d": np.asarray(bg2, np.float32).reshape(8, 128).T.copy(),
        "blind": np.asarray(b_lin, np.float32).reshape(1, 1),
        "ones11d": np.ones((1, 1), np.float32),
    }

    in_maps = []
    for c in range(NCORES):
        b0, b1q = pairs[c]
        ntok0, ntok1 = int(qlen[b0]), int(qlen[b1q])
        ntok = ntok0 + ntok1
        qs = np.zeros((128, QD), dtype=np.float32)
        qs[:ntok0] = q[qstart[b0]:qstart[b0] + ntok0]
        qs[ntok0:ntok] = q[qstart[b1q]:qstart[b1q] + ntok1]
        # host-side transpose: qT[p, kc, t] = qs[t, kc*128+p]
        qT = np.ascontiguousarray(qs.T.reshape(8, 128, 128).transpose(1, 0, 2))

        # packed (token, valid-box) rows
        vs = np.zeros((ROWS, VD), dtype=np.float32)
        sel = np.zeros((128, ROWS), dtype=np.float32)
        escat = np.zeros((128, NCHK, 128), dtype=np.float32)
        mscat = np.zeros((128, NCHK, K), dtype=np.float32)
        mask128 = np.zeros((128, K), dtype=np.float32)
        r = 0
        for lq, bq in enumerate((b0, b1q)):
            vk = valid_ks[bq]
            ntk = int(qlen[bq])
            tl0 = 0 if lq == 0 else ntok0           # local token base
            vrows = v[qstart[bq]:qstart[bq] + ntk][:, vk, :]  # [ntk, nv, VD]
            nv = len(vk)
            vs[r:r + ntk * nv] = vrows.reshape(ntk * nv, VD)
            t_loc = tl0 + np.repeat(np.arange(ntk), nv)
            kbox = np.tile(vk, ntk)
            rows = np.arange(r, r + ntk * nv)
            sel[t_loc, rows] = 1.0
            gg = np.concatenate([np.full(int(lengths[bq, g]), g) for g in range(G)])
            pp = np.concatenate([np.arange(int(lengths[bq, g])) for g in range(G)])
            p_of_tok = (lq * G + gg) * ML + pp      # [ntk]
            p_rows = np.repeat(p_of_tok, nv)        # [ntk*nv]
            escat[rows % 128, rows // 128, p_rows] = 1.0
            mscat[rows % 128, rows // 128, kbox] = 1.0
            mask128[lq * G * ML:(lq + 1) * G * ML] = box_mask[bq][None, :]
            r += ntk * nv

        # fp8 quantize + chunk-major transpose: [128, 8*rcn] per chunk
        vq8 = vs.astype(E4)
        pieces = []
        r0 = 0
        for rcn in RCNS:
            blk = vq8[r0:r0 + rcn].reshape(rcn, 8, 128).transpose(2, 1, 0)
            pieces.append(np.ascontiguousarray(blk).reshape(128, 8 * rcn))
            r0 += rcn
        vbT = np.concatenate(pieces, axis=1)

        m = dict(wb)
        m["vbT"] = vbT
        m["qTd"] = qT.astype(BF)
        m["seld"] = sel.astype(BF)
        m["maskd"] = mask128
        m["escatd"] = escat
        m["mscatd"] = mscat
        in_maps.append(m)

    if "nc" not in _CACHE:
        _CACHE["nc"] = _build_program()
    nc = _CACHE["nc"]

    LAST_RESULT = bass_utils.run_bass_kernel_spmd(
        nc, in_maps, core_ids=list(range(NCORES)))

    out = np.zeros((B, G, ML, K), dtype=np.float32)
    for c in range(NCORES):
        b0, b1q = pairs[c]
        r = LAST_RESULT.results[c]["outd"]
        out[b0] = r[:G * ML].reshape(G, ML, K)
        out[b1q] = r[G * ML:].reshape(G, ML, K)
    return out


# revision 34
# speedup vs baseline: 1.0180x; 1.0180x over previous
"""Trainium2 Bass kernel for nn_Att_PD_layer1 (ragged dual-FCNet attention logits
+ ragged pad + masked softmax), data-parallel over 8 NeuronCores.

Contract: kernel(**inputs) takes the FULL unsharded inputs and returns the FULL
[B, 4, maxlen, K] output. Sharding: 2 whole questions per core (balanced
pairing by token*valid-box rows). Only (token, valid-box) rows go through the
GEMMs.

v2: layer-1 v-GEMMs run as fp8e4 DoubleRow matmuls (2 k-tiles of 128 per
instruction; measured ~1.7x bf16 throughput on HW). Weights are
host-quantized at x64 scale; the 1/64 rescale folds into the layer-2
h-weights (leaky-relu is positively homogeneous) and into the sigmoid
drain's scale. All PSUM drains run on ACT using one activation table
(Sigmoid / Prelu / Identity share a set; Pool cannot read PSUM on HW).
v arrives host-pre-transposed fp8 in chunk-major layout (no DMA
transposes); q arrives host-pre-transposed bf16. The ragged scatter
transposes the logit row via tiny PE matmuls instead of a DRAM round trip.

Accuracy: fp8 on layer 1 only costs ~1.5e-2 rel err (gate 2e-2). Measured
dead ends: fp8 on layer 2 or on the final hg.w_lin dot blows the budget
(2.2-3.5e-2) even with split-weight compensation, because DoubleRow is only
~1.7x (not 4x) on real HW, split costs as much as it saves.
"""
import sys
import os

sys.path.insert(0, "/opt/trn_rl_repo")
# this axon env has no NTFF profiling hook; a stray BASS_TRACE=1 would crash
os.environ["BASS_NEVER_TRACE"] = "1"

import numpy as np
import ml_dtypes
from contextlib import ExitStack

import concourse.bass as bass
import concourse.tile as tile
from concourse import bacc, mybir
from concourse.bass_interp import get_hw_module
from concourse import bass_utils

F32 = mybir.dt.float32
BF16 = mybir.dt.bfloat16
FP8 = mybir.dt.float8e4
AF = mybir.ActivationFunctionType
ALU = mybir.AluOpType
DR = mybir.MatmulPerfMode.DoubleRow
BF = ml_dtypes.bfloat16
E4 = ml_dtypes.float8_e4m3fn

B, G, ML, K = 16, 4, 16, 36
VD, QD, NH = 1024, 1024, 1024
NEG_SLOPE = 0.01
SW = 64.0            # fp8 weight scale

TPC = 112                 # max tokens per core
ROWS = 1792               # max packed (token, valid-box) rows per core
RCNS = (512, 512, 512, 256)   # rows per chunk (128-multiples)
NCHK = ROWS // 128        # 14 scatter column-chunks
NCORES = 8

LAST_RESULT = None
_CACHE = {}
_TIMING_REPS = None       # when set, wraps the main body in a For_i (timing only)
_TIMING_NO_DRAIN = False  # timing probe: emit matmul stream only (garbage output)
_TIMING_UNROLL = 1        # bodies per For_i iteration (timing only)


def _build_program(pair_l2=True):
    """pair_l2=True assumes b2 == bg2 == 0 (checked by kernel()) and drains
    two mc-chunks per ACT op; the fallback path applies per-mc biases."""
    nc = bacc.Bacc("TRN2", target_bir_lowering=False, debug=False,
                   num_devices=NCORES)

    # ---- DRAM I/O (per-core shapes; same program on all 8 cores) ----
    # v pre-transposed+quantized on host: chunk-major [128, sum(8*rcn)]
    vbT = nc.dram_tensor("vbT", [128, 8 * ROWS], FP8, kind="ExternalInput")
    qTd = nc.dram_tensor("qTd", [128, 8, 128], BF16, kind="ExternalInput")
    w1q = nc.dram_tensor("w1q", [128, 8, NH], BF16, kind="ExternalInput")
    wg1q = nc.dram_tensor("wg1q", [128, 8, NH], BF16, kind="ExternalInput")
    w1v8 = nc.dram_tensor("w1v8", [128, 8, NH], FP8, kind="ExternalInput")
    wg1v8 = nc.dram_tensor("wg1v8", [128, 8, NH], FP8, kind="ExternalInput")
    # layer-2 weights in mc-major layout [p, mc, kc, 128] so each mc-slice is
    # a contiguous early DMA
    w2s = nc.dram_tensor("w2s", [128, 8, 8, 128], BF16, kind="ExternalInput")
    wg2 = nc.dram_tensor("wg2", [128, 8, 8, 128], BF16, kind="ExternalInput")
    wlin = nc.dram_tensor("wlin", [128, 8], BF16, kind="ExternalInput")
    b1x64 = nc.dram_tensor("b1x64", [128, 8], F32, kind="ExternalInput")
    bg1d = nc.dram_tensor("bg1d", [128, 8], F32, kind="ExternalInput")
    b2d = nc.dram_tensor("b2d", [128, 8], F32, kind="ExternalInput")
    bg2d = nc.dram_tensor("bg2d", [128, 8], F32, kind="ExternalInput")
    blind = nc.dram_tensor("blind", [1, 1], F32, kind="ExternalInput")
    seld = nc.dram_tensor("seld", [128, ROWS], BF16, kind="ExternalInput")
    maskd = nc.dram_tensor("maskd", [128, K], F32, kind="ExternalInput")
    escatd = nc.dram_tensor("escatd", [128, NCHK, 128], F32, kind="ExternalInput")
    mscatd = nc.dram_tensor("mscatd", [128, NCHK, K], F32, kind="ExternalInput")
    ones11d = nc.dram_tensor("ones11d", [1, 1], F32, kind="ExternalInput")

    outd = nc.dram_tensor("outd", [128, K], F32, kind="ExternalOutput")

    with tile.TileContext(nc) as tc:
        with ExitStack() as ctx:
            wpool = ctx.enter_context(tc.tile_pool(name="weights", bufs=1))
            const = ctx.enter_context(tc.tile_pool(name="const", bufs=1))
            vload = ctx.enter_context(tc.tile_pool(name="vload", bufs=2))
            acts = ctx.enter_context(tc.tile_pool(name="acts", bufs=2))
            small = ctx.enter_context(tc.tile_pool(name="small", bufs=3))
            # psum tiles are [128, 2, 512] f32 = 2 banks each; bufs=2 keeps
            # 4 matmul groups in flight (2 pairs) within 4 of the 8 banks
            psum = ctx.enter_context(tc.tile_pool(name="psum", bufs=2, space="PSUM"))
            psq = ctx.enter_context(tc.tile_pool(name="psq", bufs=2, space="PSUM"))
            pspad = ctx.enter_context(tc.tile_pool(name="pspad", bufs=1, space="PSUM"))

            def load_w(dt_, tag, dtype, halves=False):
                t = wpool.tile([128, 8, NH], dtype, tag=tag)
                if halves:
                    # two DMAs so the first U fout-half can start sooner
                    nc.gpsimd.dma_start(t[:, :, :512], dt_.ap()[:, :, :512])
                    nc.gpsimd.dma_start(t[:, :, 512:], dt_.ap()[:, :, 512:])
                else:
                    nc.gpsimd.dma_start(t[:], dt_.ap())
                return t

            # startup-critical loads first (DMA priority follows emission
            # order): U-path weights, then the fp8 layer-1 weights.
            qT = const.tile([128, 8, 128], BF16)
            nc.gpsimd.dma_start(qT[:], qTd.ap())
            w1q_t = load_w(w1q, "w1q", BF16, halves=True)
            w1v_t = load_w(w1v8, "w1v8", FP8)
            wg1q_t = load_w(wg1q, "wg1q", BF16, halves=True)
            wg1v_t = load_w(wg1v8, "wg1v8", FP8)

            wlin_t = const.tile([128, 8], BF16)
            nc.gpsimd.dma_start(wlin_t[:], wlin.ap())
            b1x64_t = const.tile([128, 8], F32)
            nc.gpsimd.dma_start(b1x64_t[:], b1x64.ap())
            bg1_t = const.tile([128, 8], F32)
            nc.gpsimd.dma_start(bg1_t[:], bg1d.ap())
            b2_t = const.tile([128, 8], F32)
            nc.gpsimd.dma_start(b2_t[:], b2d.ap())
            bg2_t = const.tile([128, 8], F32)
            nc.gpsimd.dma_start(bg2_t[:], bg2d.ap())
            blin_t = const.tile([1, 1], F32)
            nc.gpsimd.dma_start(blin_t[:], blind.ap())
            ones11_t = const.tile([1, 1], F32)
            nc.gpsimd.dma_start(ones11_t[:], ones11d.ap())
            sel_t = const.tile([128, ROWS], BF16)
            nc.gpsimd.dma_start(sel_t[:], seld.ap())

            # ---- U = (q @ Wq + b) * 64  [t(128 part), fout(1024)] bf16
            def compute_u(name, wq, bias_t):
                ut = const.tile([128, NH], BF16, tag=f"U{name}")
                for nchunk in range(2):
                    ps = psq.tile([128, 512], F32, tag="aux")
                    for kc in range(8):
                        nc.tensor.matmul(
                            ps[:], qT[:, kc, :],
                            wq[:, kc, nchunk * 512:(nchunk + 1) * 512],
                            start=(kc == 0), stop=(kc == 7))
                    if bias_t is None:
                        nc.scalar.activation(
                            ut[:, nchunk * 512:(nchunk + 1) * 512], ps[:],
                            AF.Identity, scale=SW)
                    else:
                        for j in range(4):
                            c = nchunk * 4 + j
                            nc.scalar.activation(
                                ut[:, c * 128:(c + 1) * 128],
                                ps[:, j * 128:(j + 1) * 128],
                                AF.Identity, scale=SW, bias=bias_t[:, c:c + 1])
                return ut

            # bg1 folds into Ug (x64) just like b1 into Uh, so neither l1
            # drain needs a per-mc bias and drains can pair across mc.
            U = {"h": compute_u("h", w1q_t, b1x64_t),
                 "g": compute_u("g", wg1q_t, bg1_t)}

            # logits row accumulator [1, ROWS] f32 (partition 0)
            lrow = const.tile([1, ROWS], F32)
            padded_ps = pspad.tile([128, K], F32, tag="padded")

            late = {}

            def late_loads():
                # mc-major: one DMA per mc-slice so l2-g mc0 can start early
                for nm, dt_ in (("wg2", wg2), ("w2", w2s)):
                    t = wpool.tile([128, 8, 8, 128], BF16, tag=nm)
                    for mc in range(8):
                        nc.gpsimd.dma_start(t[:, mc, :, :], dt_.ap()[:, mc, :, :])
                    late[nm] = t

            def tail_loads():
                late["escat"] = const.tile([128, NCHK, 128], F32, tag="escat_t",
                                           name="escat_t")
                nc.gpsimd.dma_start(late["escat"][:], escatd.ap())
                late["mscat"] = const.tile([128, NCHK, K], F32, tag="mscat_t",
                                           name="mscat_t")
                nc.gpsimd.dma_start(late["mscat"][:], mscatd.ap())
                late["mask"] = const.tile([128, K], F32, tag="mask_t",
                                          name="mask_t")
                nc.gpsimd.dma_start(late["mask"][:], maskd.ap())

            def main_body():
                r0 = 0
                off = 0          # element offset into vbT (chunk-major)
                for rc, rcn in enumerate(RCNS):
                    vT = vload.tile([128, 8, 512], FP8)
                    nc.sync.dma_start(
                        vT[:, :, :rcn],
                        vbT.ap()[:, off:off + 8 * rcn].rearrange(
                            "p (kc r) -> p kc r", kc=8))

                    h1T = acts.tile([128, 8, 512], BF16, tag="h1T")
                    g1T = acts.tile([128, 8, 512], BF16, tag="g1T")
                    hgT = acts.tile([128, 8, 512], BF16, tag="hgT")

                    # layer 1: 4 fp8 DoubleRow matmuls + bf16 sel/U matmul,
                    # two mc-chunks share one [128, 2, 512] psum tile so one
                    # ACT op drains both (biases live in U; none needed here).
                    def l1_mm(wv, uname, mc, psv):
                        for kp in range(4):
                            nc.tensor.matmul(
                                psv,
                                wv[:, 2 * kp:2 * kp + 2, mc * 128:(mc + 1) * 128],
                                vT[:, 2 * kp:2 * kp + 2, :rcn],
                                start=(kp == 0), stop=False, perf_mode=DR,
                                skip_group_check=True)
                        nc.tensor.matmul(
                            psv, U[uname][:, mc * 128:(mc + 1) * 128],
                            sel_t[:, r0:r0 + rcn], start=False, stop=True,
                            skip_group_check=True)

                    # h branch: paired parametric-relu drains on ACT (64x
                    # scale stays -- prelu is positively homogeneous; b1
                    # already folded into U-h; prelu shares the sigmoid table)
                    for mp in range(4):
                        ps = psum.tile([128, 2, 512], F32, tag="ps")
                        for j in range(2):
                            l1_mm(w1v_t, "h", 2 * mp + j, ps[:, j, :rcn])
                        if not _TIMING_NO_DRAIN:
                            nc.scalar.activation(
                                h1T[:, 2 * mp:2 * mp + 2, :rcn],
                                ps[:, :, :rcn], AF.Prelu, alpha=NEG_SLOPE)
                    # g branch: paired sigmoid drains (scale 1/64; bg1 in Ug)
                    for mp in range(4):
                        ps = psum.tile([128, 2, 512], F32, tag="ps")
                        for j in range(2):
                            l1_mm(wg1v_t, "g", 2 * mp + j, ps[:, j, :rcn])
                        if not _TIMING_NO_DRAIN:
                            nc.scalar.activation(
                                g1T[:, 2 * mp:2 * mp + 2, :rcn],
                                ps[:, :, :rcn], AF.Sigmoid, scale=1.0 / SW)

                    if rc == 0 and "w2" not in late:
                        late_loads()
                    w2_t, wg2_t = late["w2"], late["wg2"]

                    # layer 2 matmul group for one mc into a [128, rcn] psum view
                    def l2_mm(wt, srcT, mc, psv):
                        for kc in range(8):
                            rhs = (sel_t[:, kc * 128:kc * 128 + rcn]
                                   if _TIMING_NO_DRAIN else srcT[:, kc, :rcn])
                            nc.tensor.matmul(
                                psv, wt[:, mc, kc, :],
                                rhs, start=(kc == 0), stop=(kc == 7),
                                skip_group_check=True)

                    g2T = small.tile([128, 8, 512], BF16, tag="g2T")
                    if pair_l2:
                        # b2 == bg2 == 0: paired drains, no per-mc bias needed
                        for mp in range(4):
                            ps = psum.tile([128, 2, 512], F32, tag="ps")
                            for j in range(2):
                                l2_mm(wg2_t, g1T, 2 * mp + j, ps[:, j, :rcn])
                            if not _TIMING_NO_DRAIN:
                                nc.scalar.activation(
                                    g2T[:, 2 * mp:2 * mp + 2, :rcn],
                                    ps[:, :, :rcn], AF.Sigmoid)
                        for mp in range(4):
                            ps = psum.tile([128, 2, 512], F32, tag="ps")
                            for j in range(2):
                                l2_mm(w2_t, h1T, 2 * mp + j, ps[:, j, :rcn])
                            h2t = small.tile([128, 2, 512], BF16, tag="h2t")
                            if not _TIMING_NO_DRAIN:
                                nc.scalar.activation(h2t[:, :, :rcn],
                                                     ps[:, :, :rcn], AF.Prelu,
                                                     alpha=NEG_SLOPE)
                                nc.vector.tensor_mul(
                                    hgT[:, 2 * mp:2 * mp + 2, :rcn],
                                    h2t[:, :, :rcn],
                                    g2T[:, 2 * mp:2 * mp + 2, :rcn])
                    else:
                        # general path: per-mc biased drains
                        for mc in range(8):
                            ps = psum.tile([128, 2, 512], F32, tag="ps")
                            l2_mm(wg2_t, g1T, mc, ps[:, 0, :rcn])
                            if not _TIMING_NO_DRAIN:
                                nc.scalar.activation(g2T[:, mc, :rcn],
                                                     ps[:, 0, :rcn], AF.Sigmoid,
                                                     bias=bg2_t[:, mc:mc + 1])
                        for mc in range(8):
                            ps = psum.tile([128, 2, 512], F32, tag="ps")
                            l2_mm(w2_t, h1T, mc, ps[:, 0, :rcn])
                            h2t = small.tile([128, 2, 512], BF16, tag="h2t")
                            if not _TIMING_NO_DRAIN:
                                nc.scalar.activation(h2t[:, 0, :rcn],
                                                     ps[:, 0, :rcn], AF.Prelu,
                                                     bias=b2_t[:, mc:mc + 1],
                                                     alpha=NEG_SLOPE)
                                nc.vector.tensor_mul(hgT[:, mc, :rcn],
                                                     h2t[:, 0, :rcn],
                                                     g2T[:, mc, :rcn])

                    # final: logits[r] = hg[:, r] . wlin + blin
                    psl = psq.tile([1, 512], F32, tag="aux")
                    for kc in range(8):
                        rhs = (sel_t[:, kc * 128:kc * 128 + rcn]
                               if _TIMING_NO_DRAIN else hgT[:, kc, :rcn])
                        nc.tensor.matmul(psl[:, :rcn], wlin_t[:, kc:kc + 1],
                                         rhs, start=(kc == 0),
                                         stop=(kc == 7))
                    if not _TIMING_NO_DRAIN:
                        nc.vector.tensor_scalar_add(lrow[:, r0:r0 + rcn],
                                                    psl[:, :rcn], blin_t[:, 0:1])

                    if rc == 0 and "escat" not in late:
                        tail_loads()
                    escat_t, mscat_t, mask_t = (late["escat"], late["mscat"],
                                                late["mask"])

                    # incremental ragged scatter of finished 128-blocks:
                    # transpose lrow pieces to partitions via 1-wide matmuls,
                    # then 0/1 matmuls accumulate into padded_ps.
                    c0, ncc = r0 // 128, rcn // 128
                    r0 += rcn
                    off += 8 * rcn
                    if _TIMING_NO_DRAIN:
                        continue
                    scps = pspad.tile([128, 4], F32, tag="scps")
                    for cc in range(ncc):
                        c = c0 + cc
                        nc.tensor.matmul(
                            scps[:, cc:cc + 1],
                            lrow[0:1, c * 128:(c + 1) * 128], ones11_t[:],
                            start=True, stop=True, skip_group_check=True)
                        rhs_c = small.tile([128, K], F32, tag="rhs_c")
                        nc.vector.tensor_scalar_mul(rhs_c[:], mscat_t[:, c, :],
                                                    scps[:, cc:cc + 1])
                        nc.tensor.matmul(padded_ps[:], escat_t[:, c, :], rhs_c[:],
                                         start=(c == 0), stop=(c == NCHK - 1),
                                         skip_group_check=True)

                if _TIMING_NO_DRAIN:
                    outt = small.tile([128, K], F32, tag="outt")
                    nc.vector.memset(outt[:], 0.0)
                    nc.sync.dma_start(outd.ap(), outt[:])
                    return
                # ---- masked softmax tail (f32, exact reference semantics)
                vecm = small.tile([128, K], F32, tag="vecm")
                nc.vector.tensor_mul(vecm[:], padded_ps[:], mask_t[:])
                negmx = small.tile([128, 1], F32, tag="negmx")
                nc.vector.reduce_max(negmx[:], vecm[:], axis=mybir.AxisListType.X,
                                     negate=True)
                e = small.tile([128, K], F32, tag="e")
                nc.scalar.activation(e[:], vecm[:], AF.Exp, bias=negmx[:])
                z = small.tile([128, 1], F32, tag="z")
                nc.vector.reduce_sum(z[:], e[:], axis=mybir.AxisListType.X)
                em = small.tile([128, K], F32, tag="em")
                nc.vector.tensor_mul(em[:], e[:], mask_t[:])
                s2 = small.tile([128, 1], F32, tag="s2")
                nc.vector.reduce_sum(s2[:], em[:], axis=mybir.AxisListType.X)
                den = small.tile([128, 1], F32, tag="den")
                nc.vector.tensor_scalar_mul(den[:], z[:], 1e-13)
                nc.vector.tensor_add(den[:], den[:], s2[:])
                rec = small.tile([128, 1], F32, tag="rec")
                nc.vector.reciprocal(rec[:], den[:])
                outt = small.tile([128, K], F32, tag="outt")
                nc.vector.tensor_scalar_mul(outt[:], em[:], rec[:])
                nc.sync.dma_start(outd.ap(), outt[:])

            if _TIMING_REPS:
                late_loads()
                tail_loads()
                with tc.For_i(0, _TIMING_REPS, 1):
                    for _ in range(_TIMING_UNROLL):
                        main_body()
            else:
                main_body()

    nc.compile()
    nc.m = get_hw_module(nc.m)
    return nc


def _pair_questions(weight):
    """Greedy balanced pairing: sort desc, pair largest with smallest."""
    order = np.argsort(-np.asarray(weight), kind="stable")
    pairs = []
    lo, hi = 0, len(order) - 1
    while lo < hi:
        pairs.append((int(order[lo]), int(order[hi])))
        lo += 1
        hi -= 1
    return pairs


def _rearrange_w(w):
    """[1024, NH] -> [128, 8, NH] with fin = kc*128 + p."""
    return np.ascontiguousarray(w.reshape(8, 128, NH).transpose(1, 0, 2))


def _rearrange_w_mc(w):
    """[1024, NH] -> [128, 8mc, 8kc, 128] with fin = kc*128+p, fout = mc*128+f."""
    return np.ascontiguousarray(
        w.reshape(8, 128, 8, 128).transpose(1, 2, 0, 3))


def kernel(v, q, box_mask, tags_attention, W1, b1, W2, b2, Wg1, bg1, Wg2, bg2,
           w_lin, b_lin):
    global LAST_RESULT
    v = np.asarray(v, dtype=np.float32)
    q = np.asarray(q, dtype=np.float32)
    box_mask = np.asarray(box_mask, dtype=np.float32)
    tags_attention = np.asarray(tags_attention)

    lengths = tags_attention.sum(-1).astype(np.int64)          # [B, G]
    qlen = lengths.sum(-1)                                     # [B]
    qstart = np.concatenate([[0], np.cumsum(qlen)[:-1]])
    valid_ks = [np.where(box_mask[b] > 0)[0] for b in range(B)]
    nval = np.array([len(vk) for vk in valid_ks])
    pairs = _pair_questions(qlen * nval)
    assert len(pairs) == NCORES
    assert max(qlen[a] + qlen[b] for a, b in pairs) <= TPC
    assert max(qlen[a] * nval[a] + qlen[b] * nval[b] for a, b in pairs) <= ROWS

    W1 = np.asarray(W1, np.float32)
    Wg1 = np.asarray(Wg1, np.float32)
    W2 = np.asarray(W2, np.float32)
    Wg2 = np.asarray(Wg2, np.float32)

    # shared (per-core identical) tensors
    wb = {
        "w1q": _rearrange_w(W1[VD:]).astype(BF),
        "wg1q": _rearrange_w(Wg1[VD:]).astype(BF),
        "w1v8": _rearrange_w(W1[:VD] * SW).astype(E4),
        "wg1v8": _rearrange_w(Wg1[:VD] * SW).astype(E4),
        "w2s": _rearrange_w_mc(W2 / SW).astype(BF),
        "wg2": _rearrange_w_mc(Wg2).astype(BF),
        "wlin": np.asarray(w_lin).reshape(8, 128).T.copy().astype(BF),
        "b1x64": (np.asarray(b1, np.float32) * SW).reshape(8, 128).T.copy(),
        "bg1d": (np.asarray(bg1, np.float32) * SW).reshape(8, 128).T.copy(),
        "b2d": np.asarray(b2, np.float32).reshape(8, 128).T.copy(),
        "bg2d": np.asarray(bg2, np.float32).reshape(8, 128).T.copy(),
        "blind": np.asarray(b_lin, np.float32).reshape(1, 1),
        "ones11d": np.ones((1, 1), np.float32),
    }

    in_maps = []
    for c in range(NCORES):
        b0, b1q = pairs[c]
        ntok0, ntok1 = int(qlen[b0]), int(qlen[b1q])
        ntok = ntok0 + ntok1
        qs = np.zeros((128, QD), dtype=np.float32)
        qs[:ntok0] = q[qstart[b0]:qstart[b0] + ntok0]
        qs[ntok0:ntok] = q[qstart[b1q]:qstart[b1q] + ntok1]
        # host-side transpose: qT[p, kc, t] = qs[t, kc*128+p]
        qT = np.ascontiguousarray(qs.T.reshape(8, 128, 128).transpose(1, 0, 2))

        # packed (token, valid-box) rows
        vs = np.zeros((ROWS, VD), dtype=np.float32)
        sel = np.zeros((128, ROWS), dtype=np.float32)
        escat = np.zeros((128, NCHK, 128), dtype=np.float32)
        mscat = np.zeros((128, NCHK, K), dtype=np.float32)
        mask128 = np.zeros((128, K), dtype=np.float32)
        r = 0
        for lq, bq in enumerate((b0, b1q)):
            vk = valid_ks[bq]
            ntk = int(qlen[bq])
            tl0 = 0 if lq == 0 else ntok0           # local token base
            vrows = v[qstart[bq]:qstart[bq] + ntk][:, vk, :]  # [ntk, nv, VD]
            nv = len(vk)
            vs[r:r + ntk * nv] = vrows.reshape(ntk * nv, VD)
            t_loc = tl0 + np.repeat(np.arange(ntk), nv)
            kbox = np.tile(vk, ntk)
            rows = np.arange(r, r + ntk * nv)
            sel[t_loc, rows] = 1.0
            gg = np.concatenate([np.full(int(lengths[bq, g]), g) for g in range(G)])
            pp = np.concatenate([np.arange(int(lengths[bq, g])) for g in range(G)])
            p_of_tok = (lq * G + gg) * ML + pp      # [ntk]
            p_rows = np.repeat(p_of_tok, nv)        # [ntk*nv]
            escat[rows % 128, rows // 128, p_rows] = 1.0
            mscat[rows % 128, rows // 128, kbox] = 1.0
            mask128[lq * G * ML:(lq + 1) * G * ML] = box_mask[bq][None, :]
            r += ntk * nv

        # fp8 quantize + chunk-major transpose: [128, 8*rcn] per chunk
        vq8 = vs.astype(E4)
        pieces = []
        r0 = 0
        for rcn in RCNS:
            blk = vq8[r0:r0 + rcn].reshape(rcn, 8, 128).transpose(2, 1, 0)
            pieces.append(np.ascontiguousarray(blk).reshape(128, 8 * rcn))
            r0 += rcn
        vbT = np.concatenate(pieces, axis=1)

        m = dict(wb)
        m["vbT"] = vbT
        m["qTd"] = qT.astype(BF)
        m["seld"] = sel.astype(BF)
        m["maskd"] = mask128
        m["escatd"] = escat
        m["mscatd"] = mscat
        in_maps.append(m)

    pair_l2 = bool(np.all(np.asarray(b2) == 0) and np.all(np.asarray(bg2) == 0))
    key = ("nc", pair_l2)
    if key not in _CACHE:
        _CACHE[key] = _build_program(pair_l2)
    nc = _CACHE[key]

    LAST_RESULT = bass_utils.run_bass_kernel_spmd(
        nc, in_maps, core_ids=list(range(NCORES)))

    out = np.zeros((B, G, ML, K), dtype=np.float32)
    for c in range(NCORES):
        b0, b1q = pairs[c]
        r = LAST_RESULT.results[c]["outd"]
        out[b0] = r[:G * ML].reshape(G, ML, K)
        out[b1q] = r[G * ML:].reshape(G, ML, K)
    return out


# revision 43
# speedup vs baseline: 1.0226x; 1.0045x over previous
"""Trainium2 Bass kernel for nn_Att_PD_layer1 (ragged dual-FCNet attention logits
+ ragged pad + masked softmax), data-parallel over 8 NeuronCores.

Contract: kernel(**inputs) takes the FULL unsharded inputs and returns the FULL
[B, 4, maxlen, K] output. Sharding: 2 whole questions per core (balanced
pairing by token*valid-box rows). Only (token, valid-box) rows go through the
GEMMs.

v2: layer-1 v-GEMMs run as fp8e4 DoubleRow matmuls (2 k-tiles of 128 per
instruction; measured ~1.7x bf16 throughput on HW). Weights are
host-quantized at x64 scale; the 1/64 rescale folds into the layer-2
h-weights (leaky-relu is positively homogeneous) and into the sigmoid
drain's scale. All PSUM drains run on ACT using one activation table
(Sigmoid / Prelu / Identity share a set; Pool cannot read PSUM on HW).
v arrives host-pre-transposed fp8 in chunk-major layout (no DMA
transposes); q arrives host-pre-transposed bf16. The ragged scatter
transposes the logit row via tiny PE matmuls instead of a DRAM round trip.

Accuracy: fp8 on layer 1 only costs ~1.5e-2 rel err (gate 2e-2). Measured
dead ends: fp8 on layer 2 or on the final hg.w_lin dot blows the budget
(2.2-3.5e-2) even with split-weight compensation, because DoubleRow is only
~1.7x (not 4x) on real HW, split costs as much as it saves.
"""
import sys
import os

sys.path.insert(0, "/opt/trn_rl_repo")
# this axon env has no NTFF profiling hook; a stray BASS_TRACE=1 would crash
os.environ["BASS_NEVER_TRACE"] = "1"

import numpy as np
import ml_dtypes
from contextlib import ExitStack

import concourse.bass as bass
import concourse.tile as tile
from concourse import bacc, mybir
from concourse.bass_interp import get_hw_module
from concourse import bass_utils

F32 = mybir.dt.float32
BF16 = mybir.dt.bfloat16
FP8 = mybir.dt.float8e4
AF = mybir.ActivationFunctionType
ALU = mybir.AluOpType
DR = mybir.MatmulPerfMode.DoubleRow
BF = ml_dtypes.bfloat16
E4 = ml_dtypes.float8_e4m3fn

B, G, ML, K = 16, 4, 16, 36
VD, QD, NH = 1024, 1024, 1024
NEG_SLOPE = 0.01
SW = 64.0            # fp8 weight scale

TPC = 112                 # max tokens per core
ROWS = 1792               # max packed (token, valid-box) rows per core
RCNS = (512, 512, 512, 256)   # rows per chunk (128-multiples)
NCHK = ROWS // 128        # 14 scatter column-chunks
NCORES = 8

LAST_RESULT = None
_CACHE = {}
_TIMING_REPS = None       # when set, wraps the main body in a For_i (timing only)
_TIMING_NO_DRAIN = False  # timing probe: emit matmul stream only (garbage output)
_TIMING_UNROLL = 1        # bodies per For_i iteration (timing only)


def _build_program(pair_l2=True):
    """pair_l2=True assumes b2 == bg2 == 0 (checked by kernel()) and drains
    two mc-chunks per ACT op; the fallback path applies per-mc biases."""
    nc = bacc.Bacc("TRN2", target_bir_lowering=False, debug=False,
                   num_devices=NCORES)

    # ---- DRAM I/O (per-core shapes; same program on all 8 cores) ----
    # v pre-transposed+quantized on host: chunk-major [128, sum(8*rcn)]
    vbT = nc.dram_tensor("vbT", [128, 8 * ROWS], FP8, kind="ExternalInput")
    qTd = nc.dram_tensor("qTd", [128, 8, 128], BF16, kind="ExternalInput")
    w1q = nc.dram_tensor("w1q", [128, 8, NH], BF16, kind="ExternalInput")
    wg1q = nc.dram_tensor("wg1q", [128, 8, NH], BF16, kind="ExternalInput")
    w1v8 = nc.dram_tensor("w1v8", [128, 8, NH], FP8, kind="ExternalInput")
    wg1v8 = nc.dram_tensor("wg1v8", [128, 8, NH], FP8, kind="ExternalInput")
    # layer-2 weights in mc-major layout [p, mc, kc, 128] so each mc-slice is
    # a contiguous early DMA
    w2s = nc.dram_tensor("w2s", [128, 8, 8, 128], BF16, kind="ExternalInput")
    wg2 = nc.dram_tensor("wg2", [128, 8, 8, 128], BF16, kind="ExternalInput")
    wlin = nc.dram_tensor("wlin", [128, 8], BF16, kind="ExternalInput")
    b1x64 = nc.dram_tensor("b1x64", [128, 8], F32, kind="ExternalInput")
    bg1d = nc.dram_tensor("bg1d", [128, 8], F32, kind="ExternalInput")
    b2d = nc.dram_tensor("b2d", [128, 8], F32, kind="ExternalInput")
    bg2d = nc.dram_tensor("bg2d", [128, 8], F32, kind="ExternalInput")
    blind = nc.dram_tensor("blind", [1, 1], F32, kind="ExternalInput")
    seld = nc.dram_tensor("seld", [128, ROWS], BF16, kind="ExternalInput")
    maskd = nc.dram_tensor("maskd", [128, K], F32, kind="ExternalInput")
    escatd = nc.dram_tensor("escatd", [128, NCHK, 128], F32, kind="ExternalInput")
    mscatd = nc.dram_tensor("mscatd", [128, NCHK, K], F32, kind="ExternalInput")
    ones11d = nc.dram_tensor("ones11d", [1, 1], F32, kind="ExternalInput")

    outd = nc.dram_tensor("outd", [128, K], F32, kind="ExternalOutput")

    with tile.TileContext(nc) as tc:
        with ExitStack() as ctx:
            wpool = ctx.enter_context(tc.tile_pool(name="weights", bufs=1))
            const = ctx.enter_context(tc.tile_pool(name="const", bufs=1))
            vload = ctx.enter_context(tc.tile_pool(name="vload", bufs=2))
            acts = ctx.enter_context(tc.tile_pool(name="acts", bufs=2))
            small = ctx.enter_context(tc.tile_pool(name="small", bufs=3))
            # psum tiles are [128, 2, 512] f32 = 2 banks each; bufs=2 keeps
            # 4 matmul groups in flight (2 pairs) within 4 of the 8 banks
            psum = ctx.enter_context(tc.tile_pool(name="psum", bufs=2, space="PSUM"))
            psq = ctx.enter_context(tc.tile_pool(name="psq", bufs=2, space="PSUM"))
            pspad = ctx.enter_context(tc.tile_pool(name="pspad", bufs=1, space="PSUM"))

            def load_w(dt_, tag, dtype, halves=False):
                t = wpool.tile([128, 8, NH], dtype, tag=tag)
                if halves:
                    # two DMAs so the first U fout-half can start sooner
                    nc.gpsimd.dma_start(t[:, :, :512], dt_.ap()[:, :, :512])
                    nc.gpsimd.dma_start(t[:, :, 512:], dt_.ap()[:, :, 512:])
                else:
                    nc.gpsimd.dma_start(t[:], dt_.ap())
                return t

            # startup-critical loads first (DMA priority follows emission
            # order): U-path weights, then the fp8 layer-1 weights.
            qT = const.tile([128, 8, 128], BF16)
            nc.gpsimd.dma_start(qT[:], qTd.ap())
            w1q_t = load_w(w1q, "w1q", BF16, halves=True)
            w1v_t = load_w(w1v8, "w1v8", FP8)
            wg1q_t = load_w(wg1q, "wg1q", BF16, halves=True)
            wg1v_t = load_w(wg1v8, "wg1v8", FP8)

            wlin_t = const.tile([128, 8], BF16)
            nc.gpsimd.dma_start(wlin_t[:], wlin.ap())
            b1x64_t = const.tile([128, 8], F32)
            nc.gpsimd.dma_start(b1x64_t[:], b1x64.ap())
            bg1_t = const.tile([128, 8], F32)
            nc.gpsimd.dma_start(bg1_t[:], bg1d.ap())
            b2_t = const.tile([128, 8], F32)
            nc.gpsimd.dma_start(b2_t[:], b2d.ap())
            bg2_t = const.tile([128, 8], F32)
            nc.gpsimd.dma_start(bg2_t[:], bg2d.ap())
            blin_t = const.tile([1, 1], F32)
            nc.gpsimd.dma_start(blin_t[:], blind.ap())
            ones11_t = const.tile([1, 1], F32)
            nc.gpsimd.dma_start(ones11_t[:], ones11d.ap())
            sel_t = const.tile([128, ROWS], BF16)
            nc.gpsimd.dma_start(sel_t[:], seld.ap())

            # ---- U = (q @ Wq + b) * 64  [t(128 part), fout(1024)] bf16
            def compute_u(name, wq, bias_t):
                ut = const.tile([128, NH], BF16, tag=f"U{name}")
                for nchunk in range(2):
                    ps = psq.tile([128, 512], F32, tag="aux")
                    for kc in range(8):
                        nc.tensor.matmul(
                            ps[:], qT[:, kc, :],
                            wq[:, kc, nchunk * 512:(nchunk + 1) * 512],
                            start=(kc == 0), stop=(kc == 7))
                    if bias_t is None:
                        nc.scalar.activation(
                            ut[:, nchunk * 512:(nchunk + 1) * 512], ps[:],
                            AF.Identity, scale=SW)
                    else:
                        for j in range(4):
                            c = nchunk * 4 + j
                            nc.scalar.activation(
                                ut[:, c * 128:(c + 1) * 128],
                                ps[:, j * 128:(j + 1) * 128],
                                AF.Identity, scale=SW, bias=bias_t[:, c:c + 1])
                return ut

            # bg1 folds into Ug (x64) just like b1 into Uh, so neither l1
            # drain needs a per-mc bias and drains can pair across mc.
            U = {"h": compute_u("h", w1q_t, b1x64_t),
                 "g": compute_u("g", wg1q_t, bg1_t)}

            # logits row accumulator [1, ROWS] f32 (partition 0)
            lrow = const.tile([1, ROWS], F32)
            padded_ps = pspad.tile([128, K], F32, tag="padded")

            late = {}

            def late_loads():
                # mc-major: one DMA per mc-slice so l2-g mc0 can start early
                for nm, dt_ in (("wg2", wg2), ("w2", w2s)):
                    t = wpool.tile([128, 8, 8, 128], BF16, tag=nm)
                    for mc in range(8):
                        nc.gpsimd.dma_start(t[:, mc, :, :], dt_.ap()[:, mc, :, :])
                    late[nm] = t

            def tail_loads():
                late["escat"] = const.tile([128, NCHK, 128], F32, tag="escat_t",
                                           name="escat_t")
                nc.gpsimd.dma_start(late["escat"][:], escatd.ap())
                late["mscat"] = const.tile([128, NCHK, K], F32, tag="mscat_t",
                                           name="mscat_t")
                nc.gpsimd.dma_start(late["mscat"][:], mscatd.ap())
                late["mask"] = const.tile([128, K], F32, tag="mask_t",
                                          name="mask_t")
                nc.gpsimd.dma_start(late["mask"][:], maskd.ap())

            def main_body():
                r0 = 0
                off = 0          # element offset into vbT (chunk-major)
                pending = []     # deferred scatter of the previous chunk

                # scatter phase A: transpose lrow pieces (PE) + per-piece
                # mscat*logit products (DVE); phase B: the 0/1 matmuls that
                # accumulate into padded_ps. Emitted with l1 work in between
                # so PE never waits on the DVE products.
                def emit_scatter_a(c0, ncc):
                    mscat_t = late["mscat"]
                    scps = pspad.tile([128, 4], F32, tag="scps")
                    rhs4 = small.tile([128, 4, K], F32, tag="rhs4")
                    for cc in range(ncc):
                        c = c0 + cc
                        nc.tensor.matmul(
                            scps[:, cc:cc + 1],
                            lrow[0:1, c * 128:(c + 1) * 128], ones11_t[:],
                            start=True, stop=True, skip_group_check=True)
                        nc.vector.tensor_scalar_mul(rhs4[:, cc, :],
                                                    mscat_t[:, c, :],
                                                    scps[:, cc:cc + 1])
                    return rhs4

                def emit_scatter_b(c0, ncc, rhs4):
                    escat_t = late["escat"]
                    for cc in range(ncc):
                        c = c0 + cc
                        nc.tensor.matmul(padded_ps[:], escat_t[:, c, :],
                                         rhs4[:, cc, :], start=(c == 0),
                                         stop=(c == NCHK - 1),
                                         skip_group_check=True)

                def emit_scatter(c0, ncc):
                    emit_scatter_b(c0, ncc, emit_scatter_a(c0, ncc))

                for rc, rcn in enumerate(RCNS):
                    vT = vload.tile([128, 8, 512], FP8)
                    nc.sync.dma_start(
                        vT[:, :, :rcn],
                        vbT.ap()[:, off:off + 8 * rcn].rearrange(
                            "p (kc r) -> p kc r", kc=8))

                    h1T = acts.tile([128, 8, 512], BF16, tag="h1T")
                    g1T = acts.tile([128, 8, 512], BF16, tag="g1T")
                    hgT = acts.tile([128, 8, 512], BF16, tag="hgT")

                    # layer 1: 4 fp8 DoubleRow matmuls + bf16 sel/U matmul,
                    # two mc-chunks share one [128, 2, 512] psum tile so one
                    # ACT op drains both (biases live in U; none needed here).
                    def l1_mm(wv, uname, mc, psv):
                        for kp in range(4):
                            nc.tensor.matmul(
                                psv,
                                wv[:, 2 * kp:2 * kp + 2, mc * 128:(mc + 1) * 128],
                                vT[:, 2 * kp:2 * kp + 2, :rcn],
                                start=(kp == 0), stop=False, perf_mode=DR,
                                skip_group_check=True)
                        nc.tensor.matmul(
                            psv, U[uname][:, mc * 128:(mc + 1) * 128],
                            sel_t[:, r0:r0 + rcn], start=False, stop=True,
                            skip_group_check=True)

                    # h branch: paired parametric-relu drains on ACT (64x
                    # scale stays -- prelu is positively homogeneous; b1
                    # already folded into U-h; prelu shares the sigmoid table)
                    for mp in range(4):
                        ps = psum.tile([128, 2, 512], F32, tag="ps")
                        for j in range(2):
                            l1_mm(w1v_t, "h", 2 * mp + j, ps[:, j, :rcn])
                        if not _TIMING_NO_DRAIN:
                            nc.scalar.activation(
                                h1T[:, 2 * mp:2 * mp + 2, :rcn],
                                ps[:, :, :rcn], AF.Prelu, alpha=NEG_SLOPE)
                    # previous chunk's scatter phase A: its DVE products
                    # compute while PE runs the l1-g pairs below
                    scat = None
                    if pending and not _TIMING_NO_DRAIN:
                        scat = pending.pop()
                        scat_rhs = emit_scatter_a(*scat)

                    # g branch: paired sigmoid drains (scale 1/64; bg1 in Ug)
                    for mp in range(4):
                        ps = psum.tile([128, 2, 512], F32, tag="ps")
                        for j in range(2):
                            l1_mm(wg1v_t, "g", 2 * mp + j, ps[:, j, :rcn])
                        if not _TIMING_NO_DRAIN:
                            nc.scalar.activation(
                                g1T[:, 2 * mp:2 * mp + 2, :rcn],
                                ps[:, :, :rcn], AF.Sigmoid, scale=1.0 / SW)

                    # scatter phase B: rhs products are ready by now
                    if scat is not None:
                        emit_scatter_b(*scat, scat_rhs)

                    if rc == 0 and "w2" not in late:
                        late_loads()
                    w2_t, wg2_t = late["w2"], late["wg2"]

                    # layer 2 matmul group for one mc into a [128, rcn] psum view
                    def l2_mm(wt, srcT, mc, psv):
                        for kc in range(8):
                            rhs = (sel_t[:, kc * 128:kc * 128 + rcn]
                                   if _TIMING_NO_DRAIN else srcT[:, kc, :rcn])
                            nc.tensor.matmul(
                                psv, wt[:, mc, kc, :],
                                rhs, start=(kc == 0), stop=(kc == 7),
                                skip_group_check=True)

                    g2T = small.tile([128, 8, 512], BF16, tag="g2T")
                    if pair_l2:
                        # b2 == bg2 == 0: paired drains, no per-mc bias needed
                        for mp in range(4):
                            ps = psum.tile([128, 2, 512], F32, tag="ps")
                            for j in range(2):
                                l2_mm(wg2_t, g1T, 2 * mp + j, ps[:, j, :rcn])
                            if not _TIMING_NO_DRAIN:
                                nc.scalar.activation(
                                    g2T[:, 2 * mp:2 * mp + 2, :rcn],
                                    ps[:, :, :rcn], AF.Sigmoid)
                        for mp in range(4):
                            ps = psum.tile([128, 2, 512], F32, tag="ps")
                            for j in range(2):
                                l2_mm(w2_t, h1T, 2 * mp + j, ps[:, j, :rcn])
                            h2t = small.tile([128, 2, 512], BF16, tag="h2t")
                            if not _TIMING_NO_DRAIN:
                                nc.scalar.activation(h2t[:, :, :rcn],
                                                     ps[:, :, :rcn], AF.Prelu,
                                                     alpha=NEG_SLOPE)
                                nc.vector.tensor_mul(
                                    hgT[:, 2 * mp:2 * mp + 2, :rcn],
                                    h2t[:, :, :rcn],
                                    g2T[:, 2 * mp:2 * mp + 2, :rcn])
                    else:
                        # general path: per-mc biased drains
                        for mc in range(8):
                            ps = psum.tile([128, 2, 512], F32, tag="ps")
                            l2_mm(wg2_t, g1T, mc, ps[:, 0, :rcn])
                            if not _TIMING_NO_DRAIN:
                                nc.scalar.activation(g2T[:, mc, :rcn],
                                                     ps[:, 0, :rcn], AF.Sigmoid,
                                                     bias=bg2_t[:, mc:mc + 1])
                        for mc in range(8):
                            ps = psum.tile([128, 2, 512], F32, tag="ps")
                            l2_mm(w2_t, h1T, mc, ps[:, 0, :rcn])
                            h2t = small.tile([128, 2, 512], BF16, tag="h2t")
                            if not _TIMING_NO_DRAIN:
                                nc.scalar.activation(h2t[:, 0, :rcn],
                                                     ps[:, 0, :rcn], AF.Prelu,
                                                     bias=b2_t[:, mc:mc + 1],
                                                     alpha=NEG_SLOPE)
                                nc.vector.tensor_mul(hgT[:, mc, :rcn],
                                                     h2t[:, 0, :rcn],
                                                     g2T[:, mc, :rcn])

                    # final: logits[r] = hg[:, r] . wlin + blin
                    psl = psq.tile([1, 512], F32, tag="aux")
                    for kc in range(8):
                        rhs = (sel_t[:, kc * 128:kc * 128 + rcn]
                               if _TIMING_NO_DRAIN else hgT[:, kc, :rcn])
                        nc.tensor.matmul(psl[:, :rcn], wlin_t[:, kc:kc + 1],
                                         rhs, start=(kc == 0),
                                         stop=(kc == 7))
                    if not _TIMING_NO_DRAIN:
                        nc.vector.tensor_scalar_add(lrow[:, r0:r0 + rcn],
                                                    psl[:, :rcn], blin_t[:, 0:1])

                    if rc == 0 and "escat" not in late:
                        tail_loads()
                    escat_t, mscat_t, mask_t = (late["escat"], late["mscat"],
                                                late["mask"])

                    # queue this chunk's 128-block scatter; it is emitted
                    # during the NEXT chunk's l1 phase (the last chunk's runs
                    # right before the softmax tail below)
                    pending.append((r0 // 128, rcn // 128))
                    r0 += rcn
                    off += 8 * rcn

                if pending and not _TIMING_NO_DRAIN:
                    emit_scatter(*pending.pop())

                if _TIMING_NO_DRAIN:
                    outt = small.tile([128, K], F32, tag="outt")
                    nc.vector.memset(outt[:], 0.0)
                    nc.sync.dma_start(outd.ap(), outt[:])
                    return
                # ---- masked softmax tail (f32, exact reference semantics),
                # fused: exp+sum and mask+sum use accum_out; den in one
                # two-scalar op. out_k = e_k*m_k / (sum(e*m) + 1e-13*sum(e))
                mask_t = late["mask"]
                vecm = small.tile([128, K], F32, tag="vecm")
                nc.vector.tensor_mul(vecm[:], padded_ps[:], mask_t[:])
                negmx = small.tile([128, 1], F32, tag="negmx")
                nc.vector.reduce_max(negmx[:], vecm[:], axis=mybir.AxisListType.X,
                                     negate=True)
                e = small.tile([128, K], F32, tag="e")
                z = small.tile([128, 1], F32, tag="z")
                nc.scalar.activation(e[:], vecm[:], AF.Exp, bias=negmx[:],
                                     accum_out=z[:])
                em = small.tile([128, K], F32, tag="em")
                s2 = small.tile([128, 1], F32, tag="s2")
                nc.vector.scalar_tensor_tensor(em[:], e[:], 1.0, mask_t[:],
                                               op0=ALU.mult, op1=ALU.mult,
                                               accum_out=s2[:])
                den = small.tile([128, 1], F32, tag="den")
                nc.vector.tensor_scalar(den[:], z[:], 1e-13, s2[:],
                                        op0=ALU.mult, op1=ALU.add)
                rec = small.tile([128, 1], F32, tag="rec")
                nc.vector.reciprocal(rec[:], den[:])
                outt = small.tile([128, K], F32, tag="outt")
                nc.vector.tensor_scalar_mul(outt[:], em[:], rec[:])
                nc.sync.dma_start(outd.ap(), outt[:])

            if _TIMING_REPS:
                late_loads()
                tail_loads()
                with tc.For_i(0, _TIMING_REPS, 1):
                    for _ in range(_TIMING_UNROLL):
                        main_body()
            else:
                main_body()

    nc.compile()
    nc.m = get_hw_module(nc.m)
    return nc


def _pair_questions(weight):
    """Greedy balanced pairing: sort desc, pair largest with smallest."""
    order = np.argsort(-np.asarray(weight), kind="stable")
    pairs = []
    lo, hi = 0, len(order) - 1
    while lo < hi:
        pairs.append((int(order[lo]), int(order[hi])))
        lo += 1
        hi -= 1
    return pairs


def _rearrange_w(w):
    """[1024, NH] -> [128, 8, NH] with fin = kc*128 + p."""
    return np.ascontiguousarray(w.reshape(8, 128, NH).transpose(1, 0, 2))


def _rearrange_w_mc(w):
    """[1024, NH] -> [128, 8mc, 8kc, 128] with fin = kc*128+p, fout = mc*128+f."""
    return np.ascontiguousarray(
        w.reshape(8, 128, 8, 128).transpose(1, 2, 0, 3))


def kernel(v, q, box_mask, tags_attention, W1, b1, W2, b2, Wg1, bg1, Wg2, bg2,
           w_lin, b_lin):
    global LAST_RESULT
    v = np.asarray(v, dtype=np.float32)
    q = np.asarray(q, dtype=np.float32)
    box_mask = np.asarray(box_mask, dtype=np.float32)
    tags_attention = np.asarray(tags_attention)

    lengths = tags_attention.sum(-1).astype(np.int64)          # [B, G]
    qlen = lengths.sum(-1)                                     # [B]
    qstart = np.concatenate([[0], np.cumsum(qlen)[:-1]])
    valid_ks = [np.where(box_mask[b] > 0)[0] for b in range(B)]
    nval = np.array([len(vk) for vk in valid_ks])
    pairs = _pair_questions(qlen * nval)
    assert len(pairs) == NCORES
    assert max(qlen[a] + qlen[b] for a, b in pairs) <= TPC
    assert max(qlen[a] * nval[a] + qlen[b] * nval[b] for a, b in pairs) <= ROWS

    W1 = np.asarray(W1, np.float32)
    Wg1 = np.asarray(Wg1, np.float32)
    W2 = np.asarray(W2, np.float32)
    Wg2 = np.asarray(Wg2, np.float32)

    # shared (per-core identical) tensors
    wb = {
        "w1q": _rearrange_w(W1[VD:]).astype(BF),
        "wg1q": _rearrange_w(Wg1[VD:]).astype(BF),
        "w1v8": _rearrange_w(W1[:VD] * SW).astype(E4),
        "wg1v8": _rearrange_w(Wg1[:VD] * SW).astype(E4),
        "w2s": _rearrange_w_mc(W2 / SW).astype(BF),
        "wg2": _rearrange_w_mc(Wg2).astype(BF),
        "wlin": np.asarray(w_lin).reshape(8, 128).T.copy().astype(BF),
        "b1x64": (np.asarray(b1, np.float32) * SW).reshape(8, 128).T.copy(),
        "bg1d": (np.asarray(bg1, np.float32) * SW).reshape(8, 128).T.copy(),
        "b2d": np.asarray(b2, np.float32).reshape(8, 128).T.copy(),
        "bg2d": np.asarray(bg2, np.float32).reshape(8, 128).T.copy(),
        "blind": np.asarray(b_lin, np.float32).reshape(1, 1),
        "ones11d": np.ones((1, 1), np.float32),
    }

    in_maps = []
    for c in range(NCORES):
        b0, b1q = pairs[c]
        ntok0, ntok1 = int(qlen[b0]), int(qlen[b1q])
        ntok = ntok0 + ntok1
        qs = np.zeros((128, QD), dtype=np.float32)
        qs[:ntok0] = q[qstart[b0]:qstart[b0] + ntok0]
        qs[ntok0:ntok] = q[qstart[b1q]:qstart[b1q] + ntok1]
        # host-side transpose: qT[p, kc, t] = qs[t, kc*128+p]
        qT = np.ascontiguousarray(qs.T.reshape(8, 128, 128).transpose(1, 0, 2))

        # packed (token, valid-box) rows
        vs = np.zeros((ROWS, VD), dtype=np.float32)
        sel = np.zeros((128, ROWS), dtype=np.float32)
        escat = np.zeros((128, NCHK, 128), dtype=np.float32)
        mscat = np.zeros((128, NCHK, K), dtype=np.float32)
        mask128 = np.zeros((128, K), dtype=np.float32)
        r = 0
        for lq, bq in enumerate((b0, b1q)):
            vk = valid_ks[bq]
            ntk = int(qlen[bq])
            tl0 = 0 if lq == 0 else ntok0           # local token base
            vrows = v[qstart[bq]:qstart[bq] + ntk][:, vk, :]  # [ntk, nv, VD]
            nv = len(vk)
            vs[r:r + ntk * nv] = vrows.reshape(ntk * nv, VD)
            t_loc = tl0 + np.repeat(np.arange(ntk), nv)
            kbox = np.tile(vk, ntk)
            rows = np.arange(r, r + ntk * nv)
            sel[t_loc, rows] = 1.0
            gg = np.concatenate([np.full(int(lengths[bq, g]), g) for g in range(G)])
            pp = np.concatenate([np.arange(int(lengths[bq, g])) for g in range(G)])
            p_of_tok = (lq * G + gg) * ML + pp      # [ntk]
            p_rows = np.repeat(p_of_tok, nv)        # [ntk*nv]
            escat[rows % 128, rows // 128, p_rows] = 1.0
            mscat[rows % 128, rows // 128, kbox] = 1.0
            mask128[lq * G * ML:(lq + 1) * G * ML] = box_mask[bq][None, :]
            r += ntk * nv

        # fp8 quantize + chunk-major transpose: [128, 8*rcn] per chunk
        vq8 = vs.astype(E4)
        pieces = []
        r0 = 0
        for rcn in RCNS:
            blk = vq8[r0:r0 + rcn].reshape(rcn, 8, 128).transpose(2, 1, 0)
            pieces.append(np.ascontiguousarray(blk).reshape(128, 8 * rcn))
            r0 += rcn
        vbT = np.concatenate(pieces, axis=1)

        m = dict(wb)
        m["vbT"] = vbT
        m["qTd"] = qT.astype(BF)
        m["seld"] = sel.astype(BF)
        m["maskd"] = mask128
        m["escatd"] = escat
        m["mscatd"] = mscat
        in_maps.append(m)

    pair_l2 = bool(np.all(np.asarray(b2) == 0) and np.all(np.asarray(bg2) == 0))
    key = ("nc", pair_l2)
    if key not in _CACHE:
        _CACHE[key] = _build_program(pair_l2)
    nc = _CACHE[key]

    LAST_RESULT = bass_utils.run_bass_kernel_spmd(
        nc, in_maps, core_ids=list(range(NCORES)))

    out = np.zeros((B, G, ML, K), dtype=np.float32)
    for c in range(NCORES):
        b0, b1q = pairs[c]
        r = LAST_RESULT.results[c]["outd"]
        out[b0] = r[:G * ML].reshape(G, ML, K)
        out[b1q] = r[G * ML:].reshape(G, ML, K)
    return out


# revision 48
# speedup vs baseline: 1.0415x; 1.0184x over previous
"""Trainium2 Bass kernel for nn_Att_PD_layer1 (ragged dual-FCNet attention logits
+ ragged pad + masked softmax), data-parallel over 8 NeuronCores.

Contract: kernel(**inputs) takes the FULL unsharded inputs and returns the FULL
[B, 4, maxlen, K] output. Sharding: 2 whole questions per core (balanced
pairing by token*valid-box rows). Only (token, valid-box) rows go through the
GEMMs.

v2: layer-1 v-GEMMs run as fp8e4 DoubleRow matmuls (2 k-tiles of 128 per
instruction; measured ~1.7x bf16 throughput on HW). Weights are
host-quantized at x64 scale; the 1/64 rescale folds into the layer-2
h-weights (leaky-relu is positively homogeneous) and into the sigmoid
drain's scale. All PSUM drains run on ACT using one activation table
(Sigmoid / Prelu / Identity share a set; Pool cannot read PSUM on HW).
v arrives host-pre-transposed fp8 in chunk-major layout (no DMA
transposes); q arrives host-pre-transposed bf16. The ragged scatter
transposes the logit row via tiny PE matmuls instead of a DRAM round trip.

Accuracy: fp8 on layer 1 only costs ~1.5e-2 rel err (gate 2e-2). Measured
dead ends: fp8 on layer 2 or on the final hg.w_lin dot blows the budget
(2.2-3.5e-2) even with split-weight compensation, because DoubleRow is only
~1.7x (not 4x) on real HW, split costs as much as it saves.
"""
import sys
import os

sys.path.insert(0, "/opt/trn_rl_repo")
# this axon env has no NTFF profiling hook; a stray BASS_TRACE=1 would crash
os.environ["BASS_NEVER_TRACE"] = "1"

import numpy as np
import ml_dtypes
from contextlib import ExitStack

import concourse.bass as bass
import concourse.tile as tile
from concourse import bacc, mybir
from concourse.bass_interp import get_hw_module
from concourse import bass_utils

F32 = mybir.dt.float32
BF16 = mybir.dt.bfloat16
FP8 = mybir.dt.float8e4
AF = mybir.ActivationFunctionType
ALU = mybir.AluOpType
DR = mybir.MatmulPerfMode.DoubleRow
BF = ml_dtypes.bfloat16
E4 = ml_dtypes.float8_e4m3fn

B, G, ML, K = 16, 4, 16, 36
VD, QD, NH = 1024, 1024, 1024
NEG_SLOPE = 0.01
SW = 64.0            # fp8 weight scale

TPC = 112                 # max tokens per core
ROWS = 1752               # max packed (token, valid-box) rows per core
RCNS = (512, 512, 512, 216)   # rows per chunk (last one partial)
NCHK = -(-ROWS // 128)    # 14 scatter column-chunks (last piece is 88 rows)
NCORES = 8

LAST_RESULT = None
_CACHE = {}
_TIMING_REPS = None       # when set, wraps the main body in a For_i (timing only)
_TIMING_NO_DRAIN = False  # timing probe: emit matmul stream only (garbage output)
_TIMING_UNROLL = 1        # bodies per For_i iteration (timing only)


def _build_program(pair_l2=True):
    """pair_l2=True assumes b2 == bg2 == 0 (checked by kernel()) and drains
    two mc-chunks per ACT op; the fallback path applies per-mc biases."""
    nc = bacc.Bacc("TRN2", target_bir_lowering=False, debug=False,
                   num_devices=NCORES)

    # ---- DRAM I/O (per-core shapes; same program on all 8 cores) ----
    # v pre-transposed+quantized on host: chunk-major [128, sum(8*rcn)]
    vbT = nc.dram_tensor("vbT", [128, 8 * ROWS], FP8, kind="ExternalInput")
    qTd = nc.dram_tensor("qTd", [128, 8, 128], BF16, kind="ExternalInput")
    w1q = nc.dram_tensor("w1q", [128, 8, NH], BF16, kind="ExternalInput")
    wg1q = nc.dram_tensor("wg1q", [128, 8, NH], BF16, kind="ExternalInput")
    w1v8 = nc.dram_tensor("w1v8", [128, 8, NH], FP8, kind="ExternalInput")
    wg1v8 = nc.dram_tensor("wg1v8", [128, 8, NH], FP8, kind="ExternalInput")
    # layer-2 weights in mc-major layout [p, mc, kc, 128] so each mc-slice is
    # a contiguous early DMA
    w2s = nc.dram_tensor("w2s", [128, 8, 8, 128], BF16, kind="ExternalInput")
    wg2 = nc.dram_tensor("wg2", [128, 8, 8, 128], BF16, kind="ExternalInput")
    wlin = nc.dram_tensor("wlin", [128, 8], BF16, kind="ExternalInput")
    b1x64 = nc.dram_tensor("b1x64", [128, 8], F32, kind="ExternalInput")
    bg1d = nc.dram_tensor("bg1d", [128, 8], F32, kind="ExternalInput")
    b2d = nc.dram_tensor("b2d", [128, 8], F32, kind="ExternalInput")
    bg2d = nc.dram_tensor("bg2d", [128, 8], F32, kind="ExternalInput")
    blind = nc.dram_tensor("blind", [1, 1], F32, kind="ExternalInput")
    seld = nc.dram_tensor("seld", [128, ROWS], BF16, kind="ExternalInput")
    maskd = nc.dram_tensor("maskd", [128, K], F32, kind="ExternalInput")
    escatd = nc.dram_tensor("escatd", [128, NCHK, 128], F32, kind="ExternalInput")
    mscatd = nc.dram_tensor("mscatd", [128, NCHK, K], F32, kind="ExternalInput")
    ones11d = nc.dram_tensor("ones11d", [1, 1], F32, kind="ExternalInput")

    outd = nc.dram_tensor("outd", [128, K], F32, kind="ExternalOutput")

    with tile.TileContext(nc) as tc:
        with ExitStack() as ctx:
            wpool = ctx.enter_context(tc.tile_pool(name="weights", bufs=1))
            const = ctx.enter_context(tc.tile_pool(name="const", bufs=1))
            vload = ctx.enter_context(tc.tile_pool(name="vload", bufs=2))
            acts = ctx.enter_context(tc.tile_pool(name="acts", bufs=2))
            small = ctx.enter_context(tc.tile_pool(name="small", bufs=3))
            # psum tiles are [128, 2, 512] f32 = 2 banks each; bufs=2 keeps
            # 4 matmul groups in flight (2 pairs) within 4 of the 8 banks
            psum = ctx.enter_context(tc.tile_pool(name="psum", bufs=2, space="PSUM"))
            psq = ctx.enter_context(tc.tile_pool(name="psq", bufs=2, space="PSUM"))
            pspad = ctx.enter_context(tc.tile_pool(name="pspad", bufs=1, space="PSUM"))

            def load_w(dt_, tag, dtype, halves=False):
                t = wpool.tile([128, 8, NH], dtype, tag=tag)
                if halves:
                    # two DMAs so the first U fout-half can start sooner
                    nc.gpsimd.dma_start(t[:, :, :512], dt_.ap()[:, :, :512])
                    nc.gpsimd.dma_start(t[:, :, 512:], dt_.ap()[:, :, 512:])
                else:
                    nc.gpsimd.dma_start(t[:], dt_.ap())
                return t

            # startup-critical loads first (DMA priority follows emission
            # order): U-path weights, then the fp8 layer-1 weights.
            qT = const.tile([128, 8, 128], BF16)
            nc.gpsimd.dma_start(qT[:], qTd.ap())
            w1q_t = load_w(w1q, "w1q", BF16, halves=True)
            w1v_t = load_w(w1v8, "w1v8", FP8)
            wg1q_t = load_w(wg1q, "wg1q", BF16, halves=True)
            wg1v_t = load_w(wg1v8, "wg1v8", FP8)

            wlin_t = const.tile([128, 8], BF16)
            nc.gpsimd.dma_start(wlin_t[:], wlin.ap())
            b1x64_t = const.tile([128, 8], F32)
            nc.gpsimd.dma_start(b1x64_t[:], b1x64.ap())
            bg1_t = const.tile([128, 8], F32)
            nc.gpsimd.dma_start(bg1_t[:], bg1d.ap())
            b2_t = const.tile([128, 8], F32)
            nc.gpsimd.dma_start(b2_t[:], b2d.ap())
            bg2_t = const.tile([128, 8], F32)
            nc.gpsimd.dma_start(bg2_t[:], bg2d.ap())
            blin_t = const.tile([1, 1], F32)
            nc.gpsimd.dma_start(blin_t[:], blind.ap())
            ones11_t = const.tile([1, 1], F32)
            nc.gpsimd.dma_start(ones11_t[:], ones11d.ap())
            sel_t = const.tile([128, ROWS], BF16)
            nc.gpsimd.dma_start(sel_t[:], seld.ap())

            # ---- U = (q @ Wq + b) * 64  [t(128 part), fout(1024)] bf16
            def compute_u(name, wq, bias_t):
                ut = const.tile([128, NH], BF16, tag=f"U{name}")
                for nchunk in range(2):
                    ps = psq.tile([128, 512], F32, tag="aux")
                    for kc in range(8):
                        nc.tensor.matmul(
                            ps[:], qT[:, kc, :],
                            wq[:, kc, nchunk * 512:(nchunk + 1) * 512],
                            start=(kc == 0), stop=(kc == 7))
                    if bias_t is None:
                        nc.scalar.activation(
                            ut[:, nchunk * 512:(nchunk + 1) * 512], ps[:],
                            AF.Identity, scale=SW)
                    else:
                        for j in range(4):
                            c = nchunk * 4 + j
                            nc.scalar.activation(
                                ut[:, c * 128:(c + 1) * 128],
                                ps[:, j * 128:(j + 1) * 128],
                                AF.Identity, scale=SW, bias=bias_t[:, c:c + 1])
                return ut

            # bg1 folds into Ug (x64) just like b1 into Uh, so neither l1
            # drain needs a per-mc bias and drains can pair across mc.
            U = {"h": compute_u("h", w1q_t, b1x64_t),
                 "g": compute_u("g", wg1q_t, bg1_t)}

            # logits row accumulator [1, ROWS] f32 (partition 0)
            lrow = const.tile([1, ROWS], F32)
            padded_ps = pspad.tile([128, K], F32, tag="padded")

            late = {}

            def late_loads():
                # mc-major: one DMA per mc-slice so l2-g mc0 can start early
                for nm, dt_ in (("wg2", wg2), ("w2", w2s)):
                    t = wpool.tile([128, 8, 8, 128], BF16, tag=nm)
                    for mc in range(8):
                        nc.gpsimd.dma_start(t[:, mc, :, :], dt_.ap()[:, mc, :, :])
                    late[nm] = t

            def tail_loads():
                late["escat"] = const.tile([128, NCHK, 128], F32, tag="escat_t",
                                           name="escat_t")
                nc.gpsimd.dma_start(late["escat"][:], escatd.ap())
                late["mscat"] = const.tile([128, NCHK, K], F32, tag="mscat_t",
                                           name="mscat_t")
                nc.gpsimd.dma_start(late["mscat"][:], mscatd.ap())
                late["mask"] = const.tile([128, K], F32, tag="mask_t",
                                          name="mask_t")
                nc.gpsimd.dma_start(late["mask"][:], maskd.ap())

            def main_body():
                r0 = 0
                off = 0          # element offset into vbT (chunk-major)
                pending = []     # deferred scatter of the previous chunk

                # scatter phase A: transpose lrow pieces (PE) + per-piece
                # mscat*logit products (DVE); phase B: the 0/1 matmuls that
                # accumulate into padded_ps. Emitted with l1 work in between
                # so PE never waits on the DVE products.
                def pieces_of(r0, rcn):
                    # (column-chunk, piece size); r0 is 128-aligned, the
                    # final piece may be partial (stale scps rows beyond it
                    # are masked by host-zeroed escat/mscat rows)
                    return [(r0 // 128 + i, min(128, rcn - 128 * i))
                            for i in range(-(-rcn // 128))]

                def emit_scatter_a(pieces):
                    mscat_t = late["mscat"]
                    scps = pspad.tile([128, 4], F32, tag="scps")
                    rhs4 = small.tile([128, 4, K], F32, tag="rhs4")
                    for cc, (c, psz) in enumerate(pieces):
                        if psz < 128:
                            # zero the tail rows first (32-aligned partition
                            # start); the matmul below overwrites [0:psz]
                            al = (psz // 32) * 32
                            nc.vector.memset(scps[al:, cc:cc + 1], 0.0)
                        nc.tensor.matmul(
                            scps[:psz, cc:cc + 1],
                            lrow[0:1, c * 128:c * 128 + psz], ones11_t[:],
                            start=True, stop=True, skip_group_check=True)
                        nc.vector.tensor_scalar_mul(rhs4[:, cc, :],
                                                    mscat_t[:, c, :],
                                                    scps[:, cc:cc + 1])
                    return rhs4

                def emit_scatter_b(pieces, rhs4):
                    escat_t = late["escat"]
                    for cc, (c, psz) in enumerate(pieces):
                        nc.tensor.matmul(padded_ps[:], escat_t[:, c, :],
                                         rhs4[:, cc, :], start=(c == 0),
                                         stop=(c == NCHK - 1),
                                         skip_group_check=True)

                def emit_scatter(pieces):
                    emit_scatter_b(pieces, emit_scatter_a(pieces))

                for rc, rcn in enumerate(RCNS):
                    vT = vload.tile([128, 8, 512], FP8)
                    nc.sync.dma_start(
                        vT[:, :, :rcn],
                        vbT.ap()[:, off:off + 8 * rcn].rearrange(
                            "p (kc r) -> p kc r", kc=8))

                    h1T = acts.tile([128, 8, 512], BF16, tag="h1T")
                    g1T = acts.tile([128, 8, 512], BF16, tag="g1T")
                    hgT = acts.tile([128, 8, 512], BF16, tag="hgT")

                    # layer 1: 4 fp8 DoubleRow matmuls + bf16 sel/U matmul,
                    # two mc-chunks share one [128, 2, 512] psum tile so one
                    # ACT op drains both (biases live in U; none needed here).
                    def l1_mm(wv, uname, mc, psv):
                        for kp in range(4):
                            nc.tensor.matmul(
                                psv,
                                wv[:, 2 * kp:2 * kp + 2, mc * 128:(mc + 1) * 128],
                                vT[:, 2 * kp:2 * kp + 2, :rcn],
                                start=(kp == 0), stop=False, perf_mode=DR,
                                skip_group_check=True)
                        nc.tensor.matmul(
                            psv, U[uname][:, mc * 128:(mc + 1) * 128],
                            sel_t[:, r0:r0 + rcn], start=False, stop=True,
                            skip_group_check=True)

                    # h branch: paired parametric-relu drains on ACT (64x
                    # scale stays -- prelu is positively homogeneous; b1
                    # already folded into U-h; prelu shares the sigmoid table)
                    for mp in range(4):
                        ps = psum.tile([128, 2, 512], F32, tag="ps")
                        for j in range(2):
                            l1_mm(w1v_t, "h", 2 * mp + j, ps[:, j, :rcn])
                        if not _TIMING_NO_DRAIN:
                            nc.scalar.activation(
                                h1T[:, 2 * mp:2 * mp + 2, :rcn],
                                ps[:, :, :rcn], AF.Prelu, alpha=NEG_SLOPE)
                    # previous chunk's scatter phase A: its DVE products
                    # compute while PE runs the l1-g pairs below
                    scat = None
                    if pending and not _TIMING_NO_DRAIN:
                        scat = pending.pop()
                        scat_rhs = emit_scatter_a(scat)

                    # g branch: paired sigmoid drains (scale 1/64; bg1 in Ug)
                    for mp in range(4):
                        ps = psum.tile([128, 2, 512], F32, tag="ps")
                        for j in range(2):
                            l1_mm(wg1v_t, "g", 2 * mp + j, ps[:, j, :rcn])
                        if not _TIMING_NO_DRAIN:
                            nc.scalar.activation(
                                g1T[:, 2 * mp:2 * mp + 2, :rcn],
                                ps[:, :, :rcn], AF.Sigmoid, scale=1.0 / SW)

                    # scatter phase B: rhs products are ready by now
                    if scat is not None:
                        emit_scatter_b(scat, scat_rhs)

                    if rc == 0 and "w2" not in late:
                        late_loads()
                    w2_t, wg2_t = late["w2"], late["wg2"]

                    # layer 2 matmul group for one mc into a [128, rcn] psum view
                    def l2_mm(wt, srcT, mc, psv):
                        for kc in range(8):
                            rhs = (sel_t[:, kc * 128:kc * 128 + rcn]
                                   if _TIMING_NO_DRAIN else srcT[:, kc, :rcn])
                            nc.tensor.matmul(
                                psv, wt[:, mc, kc, :],
                                rhs, start=(kc == 0), stop=(kc == 7),
                                skip_group_check=True)

                    g2T = small.tile([128, 8, 512], BF16, tag="g2T")
                    if pair_l2:
                        # b2 == bg2 == 0: paired drains, no per-mc bias needed
                        for mp in range(4):
                            ps = psum.tile([128, 2, 512], F32, tag="ps")
                            for j in range(2):
                                l2_mm(wg2_t, g1T, 2 * mp + j, ps[:, j, :rcn])
                            if not _TIMING_NO_DRAIN:
                                nc.scalar.activation(
                                    g2T[:, 2 * mp:2 * mp + 2, :rcn],
                                    ps[:, :, :rcn], AF.Sigmoid)
                        for mp in range(4):
                            ps = psum.tile([128, 2, 512], F32, tag="ps")
                            for j in range(2):
                                l2_mm(w2_t, h1T, 2 * mp + j, ps[:, j, :rcn])
                            h2t = small.tile([128, 2, 512], BF16, tag="h2t")
                            if not _TIMING_NO_DRAIN:
                                nc.scalar.activation(h2t[:, :, :rcn],
                                                     ps[:, :, :rcn], AF.Prelu,
                                                     alpha=NEG_SLOPE)
                                nc.vector.tensor_mul(
                                    hgT[:, 2 * mp:2 * mp + 2, :rcn],
                                    h2t[:, :, :rcn],
                                    g2T[:, 2 * mp:2 * mp + 2, :rcn])
                    else:
                        # general path: per-mc biased drains
                        for mc in range(8):
                            ps = psum.tile([128, 2, 512], F32, tag="ps")
                            l2_mm(wg2_t, g1T, mc, ps[:, 0, :rcn])
                            if not _TIMING_NO_DRAIN:
                                nc.scalar.activation(g2T[:, mc, :rcn],
                                                     ps[:, 0, :rcn], AF.Sigmoid,
                                                     bias=bg2_t[:, mc:mc + 1])
                        for mc in range(8):
                            ps = psum.tile([128, 2, 512], F32, tag="ps")
                            l2_mm(w2_t, h1T, mc, ps[:, 0, :rcn])
                            h2t = small.tile([128, 2, 512], BF16, tag="h2t")
                            if not _TIMING_NO_DRAIN:
                                nc.scalar.activation(h2t[:, 0, :rcn],
                                                     ps[:, 0, :rcn], AF.Prelu,
                                                     bias=b2_t[:, mc:mc + 1],
                                                     alpha=NEG_SLOPE)
                                nc.vector.tensor_mul(hgT[:, mc, :rcn],
                                                     h2t[:, 0, :rcn],
                                                     g2T[:, mc, :rcn])

                    # final: logits[r] = hg[:, r] . wlin + blin
                    psl = psq.tile([1, 512], F32, tag="aux")
                    for kc in range(8):
                        rhs = (sel_t[:, kc * 128:kc * 128 + rcn]
                               if _TIMING_NO_DRAIN else hgT[:, kc, :rcn])
                        nc.tensor.matmul(psl[:, :rcn], wlin_t[:, kc:kc + 1],
                                         rhs, start=(kc == 0),
                                         stop=(kc == 7))
                    if not _TIMING_NO_DRAIN:
                        nc.vector.tensor_scalar_add(lrow[:, r0:r0 + rcn],
                                                    psl[:, :rcn], blin_t[:, 0:1])

                    if rc == 0 and "escat" not in late:
                        tail_loads()
                    escat_t, mscat_t, mask_t = (late["escat"], late["mscat"],
                                                late["mask"])

                    # queue this chunk's 128-block scatter; it is emitted
                    # during the NEXT chunk's l1 phase (the last chunk's runs
                    # right before the softmax tail below)
                    pending.append(pieces_of(r0, rcn))
                    r0 += rcn
                    off += 8 * rcn

                if pending and not _TIMING_NO_DRAIN:
                    emit_scatter(pending.pop())

                if _TIMING_NO_DRAIN:
                    outt = small.tile([128, K], F32, tag="outt")
                    nc.vector.memset(outt[:], 0.0)
                    nc.sync.dma_start(outd.ap(), outt[:])
                    return
                # ---- masked softmax tail (f32, exact reference semantics),
                # fused: exp+sum and mask+sum use accum_out; den in one
                # two-scalar op. out_k = e_k*m_k / (sum(e*m) + 1e-13*sum(e))
                mask_t = late["mask"]
                vecm = small.tile([128, K], F32, tag="vecm")
                nc.vector.tensor_mul(vecm[:], padded_ps[:], mask_t[:])
                negmx = small.tile([128, 1], F32, tag="negmx")
                nc.vector.reduce_max(negmx[:], vecm[:], axis=mybir.AxisListType.X,
                                     negate=True)
                e = small.tile([128, K], F32, tag="e")
                z = small.tile([128, 1], F32, tag="z")
                nc.scalar.activation(e[:], vecm[:], AF.Exp, bias=negmx[:],
                                     accum_out=z[:])
                em = small.tile([128, K], F32, tag="em")
                s2 = small.tile([128, 1], F32, tag="s2")
                nc.vector.scalar_tensor_tensor(em[:], e[:], 1.0, mask_t[:],
                                               op0=ALU.mult, op1=ALU.mult,
                                               accum_out=s2[:])
                den = small.tile([128, 1], F32, tag="den")
                nc.vector.tensor_scalar(den[:], z[:], 1e-13, s2[:],
                                        op0=ALU.mult, op1=ALU.add)
                rec = small.tile([128, 1], F32, tag="rec")
                nc.vector.reciprocal(rec[:], den[:])
                outt = small.tile([128, K], F32, tag="outt")
                nc.vector.tensor_scalar_mul(outt[:], em[:], rec[:])
                nc.sync.dma_start(outd.ap(), outt[:])

            if _TIMING_REPS:
                late_loads()
                tail_loads()
                with tc.For_i(0, _TIMING_REPS, 1):
                    for _ in range(_TIMING_UNROLL):
                        main_body()
            else:
                main_body()

    nc.compile()
    nc.m = get_hw_module(nc.m)
    return nc


def _pair_questions(weight):
    """Greedy balanced pairing: sort desc, pair largest with smallest."""
    order = np.argsort(-np.asarray(weight), kind="stable")
    pairs = []
    lo, hi = 0, len(order) - 1
    while lo < hi:
        pairs.append((int(order[lo]), int(order[hi])))
        lo += 1
        hi -= 1
    return pairs


def _rearrange_w(w):
    """[1024, NH] -> [128, 8, NH] with fin = kc*128 + p."""
    return np.ascontiguousarray(w.reshape(8, 128, NH).transpose(1, 0, 2))


def _rearrange_w_mc(w):
    """[1024, NH] -> [128, 8mc, 8kc, 128] with fin = kc*128+p, fout = mc*128+f."""
    return np.ascontiguousarray(
        w.reshape(8, 128, 8, 128).transpose(1, 2, 0, 3))


def kernel(v, q, box_mask, tags_attention, W1, b1, W2, b2, Wg1, bg1, Wg2, bg2,
           w_lin, b_lin):
    global LAST_RESULT
    v = np.asarray(v, dtype=np.float32)
    q = np.asarray(q, dtype=np.float32)
    box_mask = np.asarray(box_mask, dtype=np.float32)
    tags_attention = np.asarray(tags_attention)

    lengths = tags_attention.sum(-1).astype(np.int64)          # [B, G]
    qlen = lengths.sum(-1)                                     # [B]
    qstart = np.concatenate([[0], np.cumsum(qlen)[:-1]])
    valid_ks = [np.where(box_mask[b] > 0)[0] for b in range(B)]
    nval = np.array([len(vk) for vk in valid_ks])
    pairs = _pair_questions(qlen * nval)
    assert len(pairs) == NCORES
    assert max(qlen[a] + qlen[b] for a, b in pairs) <= TPC
    assert max(qlen[a] * nval[a] + qlen[b] * nval[b] for a, b in pairs) <= ROWS

    W1 = np.asarray(W1, np.float32)
    Wg1 = np.asarray(Wg1, np.float32)
    W2 = np.asarray(W2, np.float32)
    Wg2 = np.asarray(Wg2, np.float32)

    # shared (per-core identical) tensors
    wb = {
        "w1q": _rearrange_w(W1[VD:]).astype(BF),
        "wg1q": _rearrange_w(Wg1[VD:]).astype(BF),
        "w1v8": _rearrange_w(W1[:VD] * SW).astype(E4),
        "wg1v8": _rearrange_w(Wg1[:VD] * SW).astype(E4),
        "w2s": _rearrange_w_mc(W2 / SW).astype(BF),
        "wg2": _rearrange_w_mc(Wg2).astype(BF),
        "wlin": np.asarray(w_lin).reshape(8, 128).T.copy().astype(BF),
        "b1x64": (np.asarray(b1, np.float32) * SW).reshape(8, 128).T.copy(),
        "bg1d": (np.asarray(bg1, np.float32) * SW).reshape(8, 128).T.copy(),
        "b2d": np.asarray(b2, np.float32).reshape(8, 128).T.copy(),
        "bg2d": np.asarray(bg2, np.float32).reshape(8, 128).T.copy(),
        "blind": np.asarray(b_lin, np.float32).reshape(1, 1),
        "ones11d": np.ones((1, 1), np.float32),
    }

    in_maps = []
    for c in range(NCORES):
        b0, b1q = pairs[c]
        ntok0, ntok1 = int(qlen[b0]), int(qlen[b1q])
        ntok = ntok0 + ntok1
        qs = np.zeros((128, QD), dtype=np.float32)
        qs[:ntok0] = q[qstart[b0]:qstart[b0] + ntok0]
        qs[ntok0:ntok] = q[qstart[b1q]:qstart[b1q] + ntok1]
        # host-side transpose: qT[p, kc, t] = qs[t, kc*128+p]
        qT = np.ascontiguousarray(qs.T.reshape(8, 128, 128).transpose(1, 0, 2))

        # packed (token, valid-box) rows
        vs = np.zeros((ROWS, VD), dtype=np.float32)
        sel = np.zeros((128, ROWS), dtype=np.float32)
        escat = np.zeros((128, NCHK, 128), dtype=np.float32)
        mscat = np.zeros((128, NCHK, K), dtype=np.float32)
        mask128 = np.zeros((128, K), dtype=np.float32)
        r = 0
        for lq, bq in enumerate((b0, b1q)):
            vk = valid_ks[bq]
            ntk = int(qlen[bq])
            tl0 = 0 if lq == 0 else ntok0           # local token base
            vrows = v[qstart[bq]:qstart[bq] + ntk][:, vk, :]  # [ntk, nv, VD]
            nv = len(vk)
            vs[r:r + ntk * nv] = vrows.reshape(ntk * nv, VD)
            t_loc = tl0 + np.repeat(np.arange(ntk), nv)
            kbox = np.tile(vk, ntk)
            rows = np.arange(r, r + ntk * nv)
            sel[t_loc, rows] = 1.0
            gg = np.concatenate([np.full(int(lengths[bq, g]), g) for g in range(G)])
            pp = np.concatenate([np.arange(int(lengths[bq, g])) for g in range(G)])
            p_of_tok = (lq * G + gg) * ML + pp      # [ntk]
            p_rows = np.repeat(p_of_tok, nv)        # [ntk*nv]
            escat[rows % 128, rows // 128, p_rows] = 1.0
            mscat[rows % 128, rows // 128, kbox] = 1.0
            mask128[lq * G * ML:(lq + 1) * G * ML] = box_mask[bq][None, :]
            r += ntk * nv

        # fp8 quantize + chunk-major transpose: [128, 8*rcn] per chunk
        vq8 = vs.astype(E4)
        pieces = []
        r0 = 0
        for rcn in RCNS:
            blk = vq8[r0:r0 + rcn].reshape(rcn, 8, 128).transpose(2, 1, 0)
            pieces.append(np.ascontiguousarray(blk).reshape(128, 8 * rcn))
            r0 += rcn
        vbT = np.concatenate(pieces, axis=1)

        m = dict(wb)
        m["vbT"] = vbT
        m["qTd"] = qT.astype(BF)
        m["seld"] = sel.astype(BF)
        m["maskd"] = mask128
        m["escatd"] = escat
        m["mscatd"] = mscat
        in_maps.append(m)

    pair_l2 = bool(np.all(np.asarray(b2) == 0) and np.all(np.asarray(bg2) == 0))
    key = ("nc", pair_l2)
    if key not in _CACHE:
        _CACHE[key] = _build_program(pair_l2)
    nc = _CACHE[key]

    LAST_RESULT = bass_utils.run_bass_kernel_spmd(
        nc, in_maps, core_ids=list(range(NCORES)))

    out = np.zeros((B, G, ML, K), dtype=np.float32)
    for c in range(NCORES):
        b0, b1q = pairs[c]
        r = LAST_RESULT.results[c]["outd"]
        out[b0] = r[:G * ML].reshape(G, ML, K)
        out[b1q] = r[G * ML:].reshape(G, ML, K)
    return out
